# revision 13
# baseline (speedup 1.0000x reference)
"""Multi-head masked attention on 8 Trainium2 NeuronCores.

Sharding: data-parallel over batch (B=2 -> 2 groups of 4 cores),
tensor-parallel over heads within a group (16 heads -> 4 heads/core).
Each core computes q/k/v projections for its 4 heads (column-sharded),
causal flash-style attention in the transposed (S^T) domain, and a
row-sharded partial o-projection. The host sums the 4 partials per
batch element and adds the output bias.

All matmul operands are bf16 (inputs converted on host, halving input
DMA); accumulation stays fp32 in PSUM. Output partials are stored
bf16 and summed in fp32 on the host.

The schedule interleaves x-transposes / projections / attention per
512-row t-block so the PE never drains: PV matmuls run one step
behind QK (software pipeline) to hide the exp latency, causal masking
and y-normalization run on the Pool engine, and the reciprocal of the
two heads' softmax denominators is fused into one DVE pass.

Self-contained: hardcodes shapes B=2, T=2048, C=1024, H=16, Dh=64.
"""

import sys

sys.path.insert(0, "/opt/trn_rl_repo")

import numpy as np

import concourse.bass as bass
import concourse.tile as tile
import concourse.mybir as mybir
from concourse import bacc
from concourse.bass import ts, ds
from concourse.masks import make_identity, make_upper_triangular

F32 = mybir.dt.float32
BF16 = mybir.dt.bfloat16
AF = mybir.ActivationFunctionType
ALU = mybir.AluOpType

NP_BF16 = mybir.dt.np(BF16)

B, T, C = 2, 2048, 1024
H, DH = 16, 64
HPC = 4            # heads per core
DQC = HPC * DH     # 256 projected dims per core
N_CORES = 8
NEG = -1.0e30

TC = T // 128    # 16 t-chunks of 128
CC = C // 128    # 8 c-chunks
TJ = T // 512    # 4 t-chunks of 512


def build_program():
    nc = bacc.Bacc("TRN2", target_bir_lowering=False, debug=False)

    xb = nc.dram_tensor("xb", [T, C], BF16, kind="ExternalInput")
    wq = nc.dram_tensor("wq", [C, DQC], BF16, kind="ExternalInput")
    wk = nc.dram_tensor("wk", [C, DQC], BF16, kind="ExternalInput")
    wv = nc.dram_tensor("wv", [C, DQC], BF16, kind="ExternalInput")
    wo = nc.dram_tensor("wo", [DQC, C], BF16, kind="ExternalInput")
    bq = nc.dram_tensor("bq", [DQC], F32, kind="ExternalInput")
    bk = nc.dram_tensor("bk", [DQC], F32, kind="ExternalInput")
    bv = nc.dram_tensor("bv", [DQC], F32, kind="ExternalInput")
    out = nc.dram_tensor("out", [T, C], BF16, kind="ExternalOutput")

    scale = 1.0 / np.sqrt(DH)

    with tile.TileContext(nc) as tc:
        with (
            tc.tile_pool(name="persist", bufs=1) as pp,
            tc.tile_pool(name="ps_s", bufs=2, space="PSUM") as ps_s,
            tc.tile_pool(name="ps_w", bufs=4, space="PSUM") as ps_w,
            tc.tile_pool(name="xin", bufs=16) as px,
            tc.tile_pool(name="xw", bufs=1) as pw,
            tc.tile_pool(name="psb", bufs=3) as pexp,
            tc.tile_pool(name="small", bufs=4) as psm,
            tc.tile_pool(name="outp", bufs=3) as pout,
        ):
            # ---- persistent sbuf tensors -------------------------------
            qT = pp.tile([128, 2, T], BF16, tag="qT")   # [p, pair, t]
            kT = pp.tile([128, 2, T], BF16, tag="kT")
            vA = pp.tile([128, TC, HPC * (DH + 1)], BF16, tag="vA")
            yT = pp.tile([128, 2, T], BF16, tag="yT")
            wo_sb = pp.tile([128, 2, C], BF16, tag="wo")
            ident = pp.tile([128, 128], BF16, tag="ident")
            bqs = pp.tile([128, 2], F32, tag="bqs")
            bks = pp.tile([128, 2], F32, tag="bks")
            bvs = pp.tile([128, DQC], F32, tag="bvs")
            xT = pw.tile([128, CC, T], BF16, tag="xT")
            wq_sb = pw.tile([128, CC, DQC], BF16, tag="wq")
            wk_sb = pw.tile([128, CC, DQC], BF16, tag="wk")
            wv_sb = pw.tile([128, CC, DQC], BF16, tag="wv")

            # constants
            make_identity(nc, ident[:])
            # tri01[s, t] = 1 where t >= s else 0: multiplicative causal
            # mask for the diagonal 128-blocks, applied post-exp on Pool
            tri01 = pp.tile([128, 128], BF16, tag="tri01")
            make_upper_triangular(nc, tri01[:], val=1.0, diag=True)
            # ones column of v_aug (denominator accumulator row)
            vA4 = vA[:].rearrange("p s (h d) -> p s h d", d=DH + 1)
            nc.gpsimd.memset(vA4[:, :, :, DH : DH + 1], 1.0)

            # biases
            nc.sync.dma_start(bqs[:], bq.ap().rearrange("(k p) -> p k", p=128))
            nc.vector.tensor_scalar_mul(bqs[:], bqs[:], scale)
            nc.sync.dma_start(bks[:], bk.ap().rearrange("(k p) -> p k", p=128))
            nc.sync.dma_start(
                bvs[0:1, :], bv.ap().rearrange("(o n) -> o n", o=1)
            )
            nc.gpsimd.partition_broadcast(bvs[:], bvs[0:1, :])

            # ---- input DMAs: x chunks for tj=0 first, then weights -----
            x_tiles = []
            for tch in range(TC):
                x_tile = px.tile([128, C], BF16, tag="x")
                nc.sync.dma_start(x_tile[:], xb.ap()[ts(tch, 128), :])
                x_tiles.append(x_tile)
                if tch == 3:
                    nc.sync.dma_start(
                        wv_sb[:], wv.ap().rearrange("(c p) d -> p c d", p=128)
                    )
                    nc.sync.dma_start(
                        wq_sb[:], wq.ap().rearrange("(c p) d -> p c d", p=128)
                    )
                    nc.sync.dma_start(
                        wk_sb[:], wk.ap().rearrange("(c p) d -> p c d", p=128)
                    )
            nc.sync.dma_start(
                wo_sb[:], wo.ap().rearrange("(k p) n -> p k n", p=128)
            )

            # ---- emission helpers --------------------------------------
            def emit_transp(tch):
                # x^T for one 128-row chunk via PE transposes
                for ccg in range(CC // 4):
                    pt = ps_w.tile([128, 512], BF16, tag="w", name="pt")
                    for q in range(4):
                        cc = 4 * ccg + q
                        nc.tensor.transpose(
                            pt[:, ts(q, 128)],
                            x_tiles[tch][:, ts(cc, 128)],
                            ident[:],
                        )
                    nc.any.tensor_copy(
                        xT[:, ds(4 * ccg, 4), ts(tch, 128)],
                        pt[:].rearrange("p (c t) -> p c t", t=128),
                    )

            def emit_qk(tj):
                # Q^T / K^T projections for one 512-col t-block, both pairs
                for hp in range(2):
                    pq = ps_w.tile([128, 512], F32, tag="w", name="pq")
                    for cc in range(CC):
                        nc.tensor.matmul(
                            pq[:],
                            wq_sb[:, cc, ts(hp, 128)],
                            xT[:, cc, ts(tj, 512)],
                            start=(cc == 0),
                            stop=(cc == CC - 1),
                        )
                    nc.vector.tensor_scalar(
                        qT[:, hp, ts(tj, 512)],
                        pq[:],
                        scale,
                        bqs[:, hp : hp + 1],
                        ALU.mult,
                        ALU.add,
                    )
                    pk = ps_w.tile([128, 512], F32, tag="w", name="pk")
                    for cc in range(CC):
                        nc.tensor.matmul(
                            pk[:],
                            wk_sb[:, cc, ts(hp, 128)],
                            xT[:, cc, ts(tj, 512)],
                            start=(cc == 0),
                            stop=(cc == CC - 1),
                        )
                    nc.vector.tensor_scalar(
                        kT[:, hp, ts(tj, 512)],
                        pk[:],
                        bks[:, hp : hp + 1],
                        None,
                        ALU.add,
                    )

            def emit_v(sc):
                # V projection (natural layout) + bias for one 128-row chunk
                pv = ps_w.tile([128, 512], F32, tag="w", name="pv")
                for cc in range(CC):
                    nc.tensor.matmul(
                        pv[:, :DQC],
                        xT[:, cc, ts(sc, 128)],
                        wv_sb[:, cc, :],
                        start=(cc == 0),
                        stop=(cc == CC - 1),
                    )
                nc.vector.tensor_tensor(
                    vA4[:, sc, :, :DH],
                    pv[:, :DQC].rearrange("p (h d) -> p h d", d=DH),
                    bvs[:].rearrange("p (h d) -> p h d", d=DH),
                    ALU.add,
                )

            def emit_att(tj):
                n_sc = 4 * (tj + 1)
                for hp in range(2):
                    hA, hB = 2 * hp, 2 * hp + 1
                    ppv_A = ps_w.tile([128, 512], F32, tag="w", name="ppv_A")
                    ppv_B = ps_w.tile([128, 512], F32, tag="w", name="ppv_B")

                    def emit_pv(item):
                        sc, off, psb = item
                        for hi, h in ((0, hA), (1, hB)):
                            ppv = ppv_A if hi == 0 else ppv_B
                            nc.tensor.matmul(
                                ppv[: DH + 1, ds(off, 512 - off)],
                                vA[:, sc, ds(h * (DH + 1), DH + 1)],
                                psb[:, ds(512 * hi + off, 512 - off)],
                                start=(sc == 0),
                                stop=(sc == n_sc - 1),
                            )

                    prev = None
                    for sc in range(n_sc):
                        kd = sc - 4 * tj  # >=0 on the causal diagonal
                        off = 128 * kd if kd > 0 else 0
                        pss = ps_s.tile([128, 1024], F32, tag="s", name="pss")
                        # QK^T for both heads of the pair, row-packed.
                        # Columns [0, off) are fully masked -> skipped.
                        for hi, (half, ppos) in enumerate(
                            [(0, (0, 0)), (512, (64, 0))]
                        ):
                            prow = slice(64 * hi, 64 * hi + 64)
                            nc.tensor.matmul(
                                pss[:, ds(half + off, 512 - off)],
                                kT[prow, hp, ts(sc, 128)],
                                qT[prow, hp, ds(512 * tj + off, 512 - off)],
                                start=True,
                                stop=True,
                                tile_position=ppos,
                            )
                        psb = pexp.tile([128, 1024], BF16, tag="p", name="psb")
                        if off == 0:
                            nc.scalar.activation(psb[:], pss[:], AF.Exp)
                        else:
                            # only the written windows (cols [0,off) of
                            # each half were skipped by the QK matmul)
                            for half in (0, 512):
                                nc.scalar.activation(
                                    psb[:, ds(half + off, 512 - off)],
                                    pss[:, ds(half + off, 512 - off)],
                                    AF.Exp,
                                )
                        if kd >= 0:
                            # causal mask on the diagonal 128-block of
                            # each head: zero p where s > t (post-exp, on
                            # the Pool engine to keep DVE off this path)
                            for half in (0, 512):
                                blk = psb[:, ds(half + off, 128)]
                                nc.gpsimd.tensor_tensor(
                                    blk, blk, tri01[:], ALU.mult
                                )
                        # PV runs one step behind QK so the PE is not
                        # blocked on the exp of the current step
                        if prev is not None:
                            emit_pv(prev)
                        prev = (sc, off, psb)
                    emit_pv(prev)

                    # normalize: y^T = Y_unnorm^T * (1/denom); the fast
                    # single-pass reciprocal (~12 bits) is far inside the
                    # bf16 error budget; broadcast runs on Pool
                    for hi, h in ((0, hA), (1, hB)):
                        ppv = ppv_A if hi == 0 else ppv_B
                        den = psm.tile([1, 512], F32, tag="den")
                        nc.vector.tensor_copy(den[:], ppv[DH : DH + 1, :])
                        rec = psm.tile([1, 512], F32, tag="rec")
                        nc.vector.reciprocal_approx_fast(rec[:], den[:])
                        recB = psm.tile([DH, 512], F32, tag="recB")
                        nc.gpsimd.partition_broadcast(recB[:], rec[:])
                        nc.vector.tensor_tensor(
                            yT[ds(64 * (h % 2), DH), h // 2, ts(tj, 512)],
                            ppv[:DH, :],
                            recB[:],
                            ALU.mult,
                        )

            def emit_o(tj):
                # o-projection for one 512-row t-block
                for tt in range(4):
                    t0 = 512 * tj + 128 * tt
                    ot = pout.tile([128, C], BF16, tag="o", name="ot")
                    for nb in range(2):
                        po = ps_w.tile([128, 512], F32, tag="w", name="po")
                        for kk in range(2):
                            nc.tensor.matmul(
                                po[:],
                                yT[:, kk, ds(t0, 128)],
                                wo_sb[:, kk, ts(nb, 512)],
                                start=(kk == 0),
                                stop=(kk == 1),
                            )
                        nc.any.tensor_copy(ot[:, ts(nb, 512)], po[:])
                    nc.sync.dma_start(out.ap()[ds(t0, 128), :], ot[:])

            # ---- schedule: interleave per t-block ----------------------
            for tch in range(4):
                emit_transp(tch)
            emit_qk(0)
            for sc in range(4):
                emit_v(sc)
            emit_att(0)
            for tj in range(1, TJ):
                for tch in range(4 * tj, 4 * tj + 4):
                    emit_transp(tch)
                emit_qk(tj)
                emit_o(tj - 1)
                for sc in range(4 * tj, 4 * tj + 4):
                    emit_v(sc)
                emit_att(tj)
            emit_o(TJ - 1)

    nc.compile()
    return nc


_CACHE = {}


def _get_program():
    if "nc" not in _CACHE:
        _CACHE["nc"] = build_program()
    return _CACHE["nc"]


def make_in_maps(x, wq, bq, wk, bk, wv, bv, wo):
    xb16 = np.asarray(x, np.float32).astype(NP_BF16)
    wqb = np.asarray(wq, np.float32).astype(NP_BF16)
    wkb = np.asarray(wk, np.float32).astype(NP_BF16)
    wvb = np.asarray(wv, np.float32).astype(NP_BF16)
    wob = np.asarray(wo, np.float32).astype(NP_BF16)
    in_maps = []
    for core in range(N_CORES):
        b, g = core // 4, core % 4
        sl = slice(g * DQC, (g + 1) * DQC)
        in_maps.append(
            {
                "xb": np.ascontiguousarray(xb16[b]),
                "wq": np.ascontiguousarray(wqb[:, sl]),
                "wk": np.ascontiguousarray(wkb[:, sl]),
                "wv": np.ascontiguousarray(wvb[:, sl]),
                "wo": np.ascontiguousarray(wob[sl, :]),
                "bq": np.ascontiguousarray(np.asarray(bq, np.float32)[sl]),
                "bk": np.ascontiguousarray(np.asarray(bk, np.float32)[sl]),
                "bv": np.ascontiguousarray(np.asarray(bv, np.float32)[sl]),
            }
        )
    return in_maps


def kernel(x, wq, bq, wk, bk, wv, bv, wo, bo):
    from concourse import bass_utils

    nc = _get_program()
    in_maps = make_in_maps(x, wq, bq, wk, bk, wv, bv, wo)
    res = bass_utils.run_bass_kernel_spmd(
        nc, in_maps, core_ids=list(range(N_CORES))
    )
    y = np.zeros((B, T, C), dtype=np.float32)
    for core in range(N_CORES):
        y[core // 4] += res.results[core]["out"].astype(np.float32)
    y += np.asarray(bo, np.float32)
    return y


# revision 17
# speedup vs baseline: 1.5589x; 1.5589x over previous
"""Multi-head masked attention on 8 Trainium2 NeuronCores.

Sharding: data-parallel over batch (B=2 -> 2 groups of 4 cores),
tensor-parallel over heads within a group (16 heads -> 4 heads/core).
Each core computes q/k/v projections for its 4 heads (column-sharded),
causal flash-style attention in the transposed (S^T) domain, and a
row-sharded partial o-projection. The host sums the 4 partials per
batch element and adds the output bias.

All matmul operands are bf16 (inputs converted on host, halving input
DMA); accumulation stays fp32 in PSUM. Output partials are stored
bf16 and summed in fp32 on the host.

The schedule interleaves x-transposes / projections / attention per
512-row t-block so the PE never drains: PV matmuls run one step
behind QK (software pipeline) to hide the exp latency, causal masking
and y-normalization run on the Pool engine, and the reciprocal of the
two heads' softmax denominators is fused into one DVE pass.

Self-contained: hardcodes shapes B=2, T=2048, C=1024, H=16, Dh=64.
"""

import sys

sys.path.insert(0, "/opt/trn_rl_repo")

import numpy as np

import concourse.bass as bass
import concourse.tile as tile
import concourse.mybir as mybir
from concourse import bacc
from concourse.bass import ts, ds
from concourse.masks import make_identity, make_upper_triangular

F32 = mybir.dt.float32
BF16 = mybir.dt.bfloat16
AF = mybir.ActivationFunctionType
ALU = mybir.AluOpType

NP_BF16 = mybir.dt.np(BF16)

B, T, C = 2, 2048, 1024
H, DH = 16, 64
HPC = 4            # heads per core
DQC = HPC * DH     # 256 projected dims per core
N_CORES = 8
NEG = -1.0e30

TC = T // 128    # 16 t-chunks of 128
CC = C // 128    # 8 c-chunks
TJ = T // 512    # 4 t-chunks of 512


def build_program():
    nc = bacc.Bacc("TRN2", target_bir_lowering=False, debug=False)

    xb = nc.dram_tensor("xb", [T, C], BF16, kind="ExternalInput")
    wq = nc.dram_tensor("wq", [C, DQC], BF16, kind="ExternalInput")
    wk = nc.dram_tensor("wk", [C, DQC], BF16, kind="ExternalInput")
    wv = nc.dram_tensor("wv", [C, DQC], BF16, kind="ExternalInput")
    wo = nc.dram_tensor("wo", [DQC, C], BF16, kind="ExternalInput")
    bq = nc.dram_tensor("bq", [DQC], F32, kind="ExternalInput")
    bk = nc.dram_tensor("bk", [DQC], F32, kind="ExternalInput")
    bv = nc.dram_tensor("bv", [DQC], F32, kind="ExternalInput")
    out = nc.dram_tensor("out", [T, C], BF16, kind="ExternalOutput")

    scale = 1.0 / np.sqrt(DH)

    with tile.TileContext(nc) as tc:
        with (
            tc.tile_pool(name="persist", bufs=1) as pp,
            tc.tile_pool(name="ps_s", bufs=2, space="PSUM") as ps_s,
            tc.tile_pool(name="ps_w", bufs=4, space="PSUM") as ps_w,
            tc.tile_pool(name="xin", bufs=16) as px,
            tc.tile_pool(name="xw", bufs=1) as pw,
            tc.tile_pool(name="psb", bufs=3) as pexp,
            tc.tile_pool(name="small", bufs=4) as psm,
            tc.tile_pool(name="outp", bufs=3) as pout,
        ):
            # ---- persistent sbuf tensors -------------------------------
            qT = pp.tile([128, 2, T], BF16, tag="qT")   # [p, pair, t]
            kT = pp.tile([128, 2, T], BF16, tag="kT")
            vA = pp.tile([128, TC, HPC * (DH + 1)], BF16, tag="vA")
            yT = pp.tile([128, 2, T], BF16, tag="yT")
            wo_sb = pp.tile([128, 2, C], BF16, tag="wo")
            ident = pp.tile([128, 128], BF16, tag="ident")
            bqs = pp.tile([128, 2], F32, tag="bqs")
            bks = pp.tile([128, 2], F32, tag="bks")
            bvs = pp.tile([128, DQC], F32, tag="bvs")
            xT = pw.tile([128, CC, T], BF16, tag="xT")
            wq_sb = pw.tile([128, CC, DQC], BF16, tag="wq")
            wk_sb = pw.tile([128, CC, DQC], BF16, tag="wk")
            wv_sb = pw.tile([128, CC, DQC], BF16, tag="wv")

            # constants
            make_identity(nc, ident[:])
            # tri01[s, t] = 1 where t >= s else 0: multiplicative causal
            # mask for the diagonal 128-blocks, applied post-exp on Pool
            tri01 = pp.tile([128, 128], BF16, tag="tri01")
            make_upper_triangular(nc, tri01[:], val=1.0, diag=True)
            # ones column of v_aug (denominator accumulator row)
            vA4 = vA[:].rearrange("p s (h d) -> p s h d", d=DH + 1)
            nc.gpsimd.memset(vA4[:, :, :, DH : DH + 1], 1.0)

            # biases
            nc.sync.dma_start(bqs[:], bq.ap().rearrange("(k p) -> p k", p=128))
            nc.vector.tensor_scalar_mul(bqs[:], bqs[:], scale)
            nc.sync.dma_start(bks[:], bk.ap().rearrange("(k p) -> p k", p=128))
            nc.sync.dma_start(
                bvs[0:1, :], bv.ap().rearrange("(o n) -> o n", o=1)
            )
            nc.gpsimd.partition_broadcast(bvs[:], bvs[0:1, :])

            # ---- input DMAs: x chunks for tj=0 first, then weights -----
            x_tiles = []
            for tch in range(TC):
                x_tile = px.tile([128, C], BF16, tag="x")
                nc.sync.dma_start(x_tile[:], xb.ap()[ts(tch, 128), :])
                x_tiles.append(x_tile)
                if tch == 3:
                    nc.sync.dma_start(
                        wv_sb[:], wv.ap().rearrange("(c p) d -> p c d", p=128)
                    )
                    nc.sync.dma_start(
                        wq_sb[:], wq.ap().rearrange("(c p) d -> p c d", p=128)
                    )
                    nc.sync.dma_start(
                        wk_sb[:], wk.ap().rearrange("(c p) d -> p c d", p=128)
                    )
            nc.sync.dma_start(
                wo_sb[:], wo.ap().rearrange("(k p) n -> p k n", p=128)
            )

            # ---- emission helpers --------------------------------------
            def emit_transp(tch):
                # x^T for one 128-row chunk via PE transposes
                for ccg in range(CC // 4):
                    pt = ps_w.tile([128, 512], BF16, tag="w", name="pt")
                    for q in range(4):
                        cc = 4 * ccg + q
                        nc.tensor.transpose(
                            pt[:, ts(q, 128)],
                            x_tiles[tch][:, ts(cc, 128)],
                            ident[:],
                        )
                    dst = xT[:, ds(4 * ccg, 4), ts(tch, 128)]
                    src = pt[:].rearrange("p (c t) -> p c t", t=128)
                    # alternate DVE/Act explicitly; never Pool (no PSUM
                    # access) and keep its ucode library stable
                    if ccg == 0:
                        nc.vector.tensor_copy(dst, src)
                    else:
                        nc.scalar.copy(dst, src)

            def emit_qk(tj):
                # Q^T / K^T projections for one 512-col t-block, both pairs
                for hp in range(2):
                    pq = ps_w.tile([128, 512], F32, tag="w", name="pq")
                    for cc in range(CC):
                        nc.tensor.matmul(
                            pq[:],
                            wq_sb[:, cc, ts(hp, 128)],
                            xT[:, cc, ts(tj, 512)],
                            start=(cc == 0),
                            stop=(cc == CC - 1),
                        )
                    nc.vector.tensor_scalar(
                        qT[:, hp, ts(tj, 512)],
                        pq[:],
                        scale,
                        bqs[:, hp : hp + 1],
                        ALU.mult,
                        ALU.add,
                    )
                    pk = ps_w.tile([128, 512], F32, tag="w", name="pk")
                    for cc in range(CC):
                        nc.tensor.matmul(
                            pk[:],
                            wk_sb[:, cc, ts(hp, 128)],
                            xT[:, cc, ts(tj, 512)],
                            start=(cc == 0),
                            stop=(cc == CC - 1),
                        )
                    nc.vector.tensor_scalar(
                        kT[:, hp, ts(tj, 512)],
                        pk[:],
                        bks[:, hp : hp + 1],
                        None,
                        ALU.add,
                    )

            def emit_v(sc):
                # V projection (natural layout) + bias for one 128-row chunk
                pv = ps_w.tile([128, 512], F32, tag="w", name="pv")
                for cc in range(CC):
                    nc.tensor.matmul(
                        pv[:, :DQC],
                        xT[:, cc, ts(sc, 128)],
                        wv_sb[:, cc, :],
                        start=(cc == 0),
                        stop=(cc == CC - 1),
                    )
                nc.vector.tensor_tensor(
                    vA4[:, sc, :, :DH],
                    pv[:, :DQC].rearrange("p (h d) -> p h d", d=DH),
                    bvs[:].rearrange("p (h d) -> p h d", d=DH),
                    ALU.add,
                )

            def emit_att(tj):
                n_sc = 4 * (tj + 1)
                for hp in range(2):
                    hA, hB = 2 * hp, 2 * hp + 1
                    ppv_A = ps_w.tile([128, 512], F32, tag="w", name="ppv_A")
                    ppv_B = ps_w.tile([128, 512], F32, tag="w", name="ppv_B")

                    def emit_pv(item):
                        sc, off, psb = item
                        for hi, h in ((0, hA), (1, hB)):
                            ppv = ppv_A if hi == 0 else ppv_B
                            nc.tensor.matmul(
                                ppv[: DH + 1, ds(off, 512 - off)],
                                vA[:, sc, ds(h * (DH + 1), DH + 1)],
                                psb[:, ds(512 * hi + off, 512 - off)],
                                start=(sc == 0),
                                stop=(sc == n_sc - 1),
                            )

                    prev = None
                    for sc in range(n_sc):
                        kd = sc - 4 * tj  # >=0 on the causal diagonal
                        off = 128 * kd if kd > 0 else 0
                        pss = ps_s.tile([128, 1024], F32, tag="s", name="pss")
                        # QK^T for both heads of the pair, row-packed.
                        # Columns [0, off) are fully masked -> skipped.
                        for hi, (half, ppos) in enumerate(
                            [(0, (0, 0)), (512, (64, 0))]
                        ):
                            prow = slice(64 * hi, 64 * hi + 64)
                            nc.tensor.matmul(
                                pss[:, ds(half + off, 512 - off)],
                                kT[prow, hp, ts(sc, 128)],
                                qT[prow, hp, ds(512 * tj + off, 512 - off)],
                                start=True,
                                stop=True,
                                tile_position=ppos,
                            )
                        psb = pexp.tile([128, 1024], BF16, tag="p", name="psb")
                        if off == 0:
                            nc.scalar.activation(psb[:], pss[:], AF.Exp)
                        else:
                            # only the written windows (cols [0,off) of
                            # each half were skipped by the QK matmul)
                            for half in (0, 512):
                                nc.scalar.activation(
                                    psb[:, ds(half + off, 512 - off)],
                                    pss[:, ds(half + off, 512 - off)],
                                    AF.Exp,
                                )
                        if kd >= 0:
                            # causal mask on the diagonal 128-block of
                            # each head: zero p where s > t (post-exp,
                            # cheap bf16 multiply on DVE; Pool only runs
                            # partition_broadcast to avoid ucode-library
                            # swaps that head-of-line block its queue)
                            for half in (0, 512):
                                blk = psb[:, ds(half + off, 128)]
                                nc.vector.tensor_tensor(
                                    blk, blk, tri01[:], ALU.mult
                                )
                        # PV runs one step behind QK so the PE is not
                        # blocked on the exp of the current step
                        if prev is not None:
                            emit_pv(prev)
                        prev = (sc, off, psb)
                    emit_pv(prev)

                    # normalize: y^T = Y_unnorm^T * (1/denom); the fast
                    # single-pass reciprocal (~18 bits) is far inside the
                    # bf16 error budget. Emission order keeps DVE from
                    # idling on the Pool broadcast round-trip.
                    recs = []
                    for hi, h in ((0, hA), (1, hB)):
                        ppv = ppv_A if hi == 0 else ppv_B
                        den = psm.tile([1, 512], F32, tag="den")
                        nc.vector.tensor_copy(den[:], ppv[DH : DH + 1, :])
                        rec = psm.tile([1, 512], F32, tag="rec")
                        nc.vector.reciprocal_approx_fast(rec[:], den[:])
                        recs.append(rec)
                    recBs = []
                    for rec in recs:
                        recB = psm.tile([DH, 512], F32, tag="recB")
                        nc.gpsimd.partition_broadcast(recB[:], rec[:])
                        recBs.append(recB)
                    for hi, h in ((0, hA), (1, hB)):
                        ppv = ppv_A if hi == 0 else ppv_B
                        nc.vector.tensor_tensor(
                            yT[ds(64 * (h % 2), DH), h // 2, ts(tj, 512)],
                            ppv[:DH, :],
                            recBs[hi][:],
                            ALU.mult,
                        )

            def emit_o(tj):
                # o-projection for one 512-row t-block
                for tt in range(4):
                    t0 = 512 * tj + 128 * tt
                    ot = pout.tile([128, C], BF16, tag="o", name="ot")
                    for nb in range(2):
                        po = ps_w.tile([128, 512], F32, tag="w", name="po")
                        for kk in range(2):
                            nc.tensor.matmul(
                                po[:],
                                yT[:, kk, ds(t0, 128)],
                                wo_sb[:, kk, ts(nb, 512)],
                                start=(kk == 0),
                                stop=(kk == 1),
                            )
                        if nb == 0:
                            nc.vector.tensor_copy(ot[:, ts(nb, 512)], po[:])
                        else:
                            nc.scalar.copy(ot[:, ts(nb, 512)], po[:])
                    nc.sync.dma_start(out.ap()[ds(t0, 128), :], ot[:])

            # ---- schedule: interleave per t-block ----------------------
            for tch in range(4):
                emit_transp(tch)
            emit_qk(0)
            for sc in range(4):
                emit_v(sc)
            emit_att(0)
            for tj in range(1, TJ):
                for tch in range(4 * tj, 4 * tj + 4):
                    emit_transp(tch)
                emit_qk(tj)
                emit_o(tj - 1)
                for sc in range(4 * tj, 4 * tj + 4):
                    emit_v(sc)
                emit_att(tj)
            emit_o(TJ - 1)

    nc.compile()
    return nc


_CACHE = {}


def _get_program():
    if "nc" not in _CACHE:
        _CACHE["nc"] = build_program()
    return _CACHE["nc"]


def make_in_maps(x, wq, bq, wk, bk, wv, bv, wo):
    xb16 = np.asarray(x, np.float32).astype(NP_BF16)
    wqb = np.asarray(wq, np.float32).astype(NP_BF16)
    wkb = np.asarray(wk, np.float32).astype(NP_BF16)
    wvb = np.asarray(wv, np.float32).astype(NP_BF16)
    wob = np.asarray(wo, np.float32).astype(NP_BF16)
    in_maps = []
    for core in range(N_CORES):
        b, g = core // 4, core % 4
        sl = slice(g * DQC, (g + 1) * DQC)
        in_maps.append(
            {
                "xb": np.ascontiguousarray(xb16[b]),
                "wq": np.ascontiguousarray(wqb[:, sl]),
                "wk": np.ascontiguousarray(wkb[:, sl]),
                "wv": np.ascontiguousarray(wvb[:, sl]),
                "wo": np.ascontiguousarray(wob[sl, :]),
                "bq": np.ascontiguousarray(np.asarray(bq, np.float32)[sl]),
                "bk": np.ascontiguousarray(np.asarray(bk, np.float32)[sl]),
                "bv": np.ascontiguousarray(np.asarray(bv, np.float32)[sl]),
            }
        )
    return in_maps


def kernel(x, wq, bq, wk, bk, wv, bv, wo, bo):
    from concourse import bass_utils

    nc = _get_program()
    in_maps = make_in_maps(x, wq, bq, wk, bk, wv, bv, wo)
    res = bass_utils.run_bass_kernel_spmd(
        nc, in_maps, core_ids=list(range(N_CORES))
    )
    y = np.zeros((B, T, C), dtype=np.float32)
    for core in range(N_CORES):
        y[core // 4] += res.results[core]["out"].astype(np.float32)
    y += np.asarray(bo, np.float32)
    return y


# revision 20
# speedup vs baseline: 1.5623x; 1.0022x over previous
"""Multi-head masked attention on 8 Trainium2 NeuronCores.

Sharding: data-parallel over batch (B=2 -> 2 groups of 4 cores),
tensor-parallel over heads within a group (16 heads -> 4 heads/core).
Each core computes q/k/v projections for its 4 heads (column-sharded),
causal flash-style attention in the transposed (S^T) domain, and a
row-sharded partial o-projection. The host sums the 4 partials per
batch element and adds the output bias.

All matmul operands are bf16 (inputs converted on host, halving input
DMA); accumulation stays fp32 in PSUM. Output partials are stored
bf16 and summed in fp32 on the host.

The schedule interleaves x-transposes / projections / attention per
512-row t-block so the PE never drains: PV matmuls run one step
behind QK (software pipeline) to hide the exp latency, causal masking
and y-normalization run on the Pool engine, and the reciprocal of the
two heads' softmax denominators is fused into one DVE pass.

Self-contained: hardcodes shapes B=2, T=2048, C=1024, H=16, Dh=64.
"""

import sys

sys.path.insert(0, "/opt/trn_rl_repo")

import numpy as np

import concourse.bass as bass
import concourse.tile as tile
import concourse.mybir as mybir
from concourse import bacc
from concourse.bass import ts, ds
from concourse.masks import make_identity, make_upper_triangular

F32 = mybir.dt.float32
BF16 = mybir.dt.bfloat16
AF = mybir.ActivationFunctionType
ALU = mybir.AluOpType

NP_BF16 = mybir.dt.np(BF16)

B, T, C = 2, 2048, 1024
H, DH = 16, 64
HPC = 4            # heads per core
DQC = HPC * DH     # 256 projected dims per core
N_CORES = 8
NEG = -1.0e30

TC = T // 128    # 16 t-chunks of 128
CC = C // 128    # 8 c-chunks
TJ = T // 512    # 4 t-chunks of 512


def build_program():
    nc = bacc.Bacc("TRN2", target_bir_lowering=False, debug=False)

    xb = nc.dram_tensor("xb", [T, C], BF16, kind="ExternalInput")
    wq = nc.dram_tensor("wq", [C, DQC], BF16, kind="ExternalInput")
    wk = nc.dram_tensor("wk", [C, DQC], BF16, kind="ExternalInput")
    wv = nc.dram_tensor("wv", [C, DQC], BF16, kind="ExternalInput")
    wo = nc.dram_tensor("wo", [DQC, C], BF16, kind="ExternalInput")
    bq = nc.dram_tensor("bq", [DQC], F32, kind="ExternalInput")
    bk = nc.dram_tensor("bk", [DQC], F32, kind="ExternalInput")
    bv = nc.dram_tensor("bv", [DQC], F32, kind="ExternalInput")
    out = nc.dram_tensor("out", [T, C], BF16, kind="ExternalOutput")

    scale = 1.0 / np.sqrt(DH)

    with tile.TileContext(nc) as tc:
        with (
            tc.tile_pool(name="persist", bufs=1) as pp,
            tc.tile_pool(name="ps_s", bufs=2, space="PSUM") as ps_s,
            tc.tile_pool(name="ps_w", bufs=4, space="PSUM") as ps_w,
            tc.tile_pool(name="xin", bufs=16) as px,
            tc.tile_pool(name="xw", bufs=1) as pw,
            tc.tile_pool(name="psb", bufs=3) as pexp,
            tc.tile_pool(name="small", bufs=4) as psm,
            tc.tile_pool(name="outp", bufs=3) as pout,
        ):
            # ---- persistent sbuf tensors -------------------------------
            qT = pp.tile([128, 2, T], BF16, tag="qT")   # [p, pair, t]
            kT = pp.tile([128, 2, T], BF16, tag="kT")
            vA = pp.tile([128, TC, HPC * (DH + 1)], BF16, tag="vA")
            yT = pp.tile([128, 2, T], BF16, tag="yT")
            wo_sb = pp.tile([128, 2, C], BF16, tag="wo")
            ident = pp.tile([128, 128], BF16, tag="ident")
            bqs = pp.tile([128, 2], F32, tag="bqs")
            bks = pp.tile([128, 2], F32, tag="bks")
            bvs = pp.tile([128, DQC], F32, tag="bvs")
            xT = pw.tile([128, CC, T], BF16, tag="xT")
            wq_sb = pw.tile([128, CC, DQC], BF16, tag="wq")
            wk_sb = pw.tile([128, CC, DQC], BF16, tag="wk")
            wv_sb = pw.tile([128, CC, DQC], BF16, tag="wv")

            # constants
            make_identity(nc, ident[:])
            # tri01[s, t] = 1 where t >= s else 0: multiplicative causal
            # mask for the diagonal 128-blocks, applied post-exp on Pool
            tri01 = pp.tile([128, 128], BF16, tag="tri01")
            make_upper_triangular(nc, tri01[:], val=1.0, diag=True)
            # ones column of v_aug (denominator accumulator row)
            vA4 = vA[:].rearrange("p s (h d) -> p s h d", d=DH + 1)
            nc.gpsimd.memset(vA4[:, :, :, DH : DH + 1], 1.0)

            # biases
            nc.sync.dma_start(bqs[:], bq.ap().rearrange("(k p) -> p k", p=128))
            nc.vector.tensor_scalar_mul(bqs[:], bqs[:], scale)
            nc.sync.dma_start(bks[:], bk.ap().rearrange("(k p) -> p k", p=128))
            nc.sync.dma_start(
                bvs[0:1, :], bv.ap().rearrange("(o n) -> o n", o=1)
            )
            nc.gpsimd.partition_broadcast(bvs[:], bvs[0:1, :])

            # ---- input DMAs: x chunks for tj=0 first, then weights -----
            x_tiles = []
            for tch in range(TC):
                x_tile = px.tile([128, C], BF16, tag="x")
                nc.sync.dma_start(x_tile[:], xb.ap()[ts(tch, 128), :])
                x_tiles.append(x_tile)
                if tch == 3:
                    nc.sync.dma_start(
                        wq_sb[:], wq.ap().rearrange("(c p) d -> p c d", p=128)
                    )
                    nc.sync.dma_start(
                        wk_sb[:], wk.ap().rearrange("(c p) d -> p c d", p=128)
                    )
                    nc.sync.dma_start(
                        wv_sb[:], wv.ap().rearrange("(c p) d -> p c d", p=128)
                    )
            nc.sync.dma_start(
                wo_sb[:], wo.ap().rearrange("(k p) n -> p k n", p=128)
            )

            # ---- emission helpers --------------------------------------
            def emit_transp(tch):
                # x^T for one 128-row chunk via PE transposes
                for ccg in range(CC // 4):
                    pt = ps_w.tile([128, 512], BF16, tag="w", name="pt")
                    for q in range(4):
                        cc = 4 * ccg + q
                        nc.tensor.transpose(
                            pt[:, ts(q, 128)],
                            x_tiles[tch][:, ts(cc, 128)],
                            ident[:],
                        )
                    dst = xT[:, ds(4 * ccg, 4), ts(tch, 128)]
                    src = pt[:].rearrange("p (c t) -> p c t", t=128)
                    # alternate DVE/Act explicitly; never Pool (no PSUM
                    # access) and keep its ucode library stable
                    if ccg == 0:
                        nc.vector.tensor_copy(dst, src)
                    else:
                        nc.scalar.copy(dst, src)

            def emit_qk(tj):
                # Q^T / K^T projections for one 512-col t-block, both pairs
                for hp in range(2):
                    pq = ps_w.tile([128, 512], F32, tag="w", name="pq")
                    for cc in range(CC):
                        nc.tensor.matmul(
                            pq[:],
                            wq_sb[:, cc, ts(hp, 128)],
                            xT[:, cc, ts(tj, 512)],
                            start=(cc == 0),
                            stop=(cc == CC - 1),
                        )
                    nc.vector.tensor_scalar(
                        qT[:, hp, ts(tj, 512)],
                        pq[:],
                        scale,
                        bqs[:, hp : hp + 1],
                        ALU.mult,
                        ALU.add,
                    )
                    pk = ps_w.tile([128, 512], F32, tag="w", name="pk")
                    for cc in range(CC):
                        nc.tensor.matmul(
                            pk[:],
                            wk_sb[:, cc, ts(hp, 128)],
                            xT[:, cc, ts(tj, 512)],
                            start=(cc == 0),
                            stop=(cc == CC - 1),
                        )
                    nc.vector.tensor_scalar(
                        kT[:, hp, ts(tj, 512)],
                        pk[:],
                        bks[:, hp : hp + 1],
                        None,
                        ALU.add,
                    )

            def emit_v(sc):
                # V projection (natural layout) + bias for one 128-row chunk
                pv = ps_w.tile([128, 512], F32, tag="w", name="pv")
                for cc in range(CC):
                    nc.tensor.matmul(
                        pv[:, :DQC],
                        xT[:, cc, ts(sc, 128)],
                        wv_sb[:, cc, :],
                        start=(cc == 0),
                        stop=(cc == CC - 1),
                    )
                nc.vector.tensor_tensor(
                    vA4[:, sc, :, :DH],
                    pv[:, :DQC].rearrange("p (h d) -> p h d", d=DH),
                    bvs[:].rearrange("p (h d) -> p h d", d=DH),
                    ALU.add,
                )

            def emit_att(tj):
                n_sc = 4 * (tj + 1)
                for hp in range(2):
                    hA, hB = 2 * hp, 2 * hp + 1
                    ppv_A = ps_w.tile([128, 512], F32, tag="w", name="ppv_A")
                    ppv_B = ps_w.tile([128, 512], F32, tag="w", name="ppv_B")

                    def emit_pv(item):
                        sc, off, psb = item
                        for hi, h in ((0, hA), (1, hB)):
                            ppv = ppv_A if hi == 0 else ppv_B
                            nc.tensor.matmul(
                                ppv[: DH + 1, ds(off, 512 - off)],
                                vA[:, sc, ds(h * (DH + 1), DH + 1)],
                                psb[:, ds(512 * hi + off, 512 - off)],
                                start=(sc == 0),
                                stop=(sc == n_sc - 1),
                            )

                    prev = None
                    for sc in range(n_sc):
                        kd = sc - 4 * tj  # >=0 on the causal diagonal
                        off = 128 * kd if kd > 0 else 0
                        # tj=0 is Act-overhead-bound: keep QK full width
                        # there so one full-width exp suffices (the extra
                        # columns are garbage-but-unread; PV trims at off)
                        qlo = off if tj > 0 else 0
                        pss = ps_s.tile([128, 1024], F32, tag="s", name="pss")
                        # QK^T for both heads of the pair, row-packed.
                        # Columns [0, qlo) are fully masked -> skipped.
                        for hi, (half, ppos) in enumerate(
                            [(0, (0, 0)), (512, (64, 0))]
                        ):
                            prow = slice(64 * hi, 64 * hi + 64)
                            nc.tensor.matmul(
                                pss[:, ds(half + qlo, 512 - qlo)],
                                kT[prow, hp, ts(sc, 128)],
                                qT[prow, hp, ds(512 * tj + qlo, 512 - qlo)],
                                start=True,
                                stop=True,
                                tile_position=ppos,
                            )
                        psb = pexp.tile([128, 1024], BF16, tag="p", name="psb")
                        if qlo == 0:
                            nc.scalar.activation(psb[:], pss[:], AF.Exp)
                        else:
                            # only the written windows (cols [0,qlo) of
                            # each half were skipped by the QK matmul)
                            for half in (0, 512):
                                nc.scalar.activation(
                                    psb[:, ds(half + qlo, 512 - qlo)],
                                    pss[:, ds(half + qlo, 512 - qlo)],
                                    AF.Exp,
                                )
                        if kd >= 0:
                            # causal mask on the diagonal 128-block of
                            # each head: zero p where s > t (post-exp,
                            # cheap bf16 multiply on DVE; Pool only runs
                            # partition_broadcast to avoid ucode-library
                            # swaps that head-of-line block its queue)
                            for half in (0, 512):
                                blk = psb[:, ds(half + off, 128)]
                                nc.vector.tensor_tensor(
                                    blk, blk, tri01[:], ALU.mult
                                )
                        # PV runs one step behind QK so the PE is not
                        # blocked on the exp of the current step
                        if prev is not None:
                            emit_pv(prev)
                        prev = (sc, off, psb)
                    emit_pv(prev)

                    # normalize: y^T = Y_unnorm^T * (1/denom); the fast
                    # single-pass reciprocal (~18 bits) is far inside the
                    # bf16 error budget. Emission order keeps DVE from
                    # idling on the Pool broadcast round-trip.
                    recs = []
                    for hi, h in ((0, hA), (1, hB)):
                        ppv = ppv_A if hi == 0 else ppv_B
                        den = psm.tile([1, 512], F32, tag="den")
                        nc.vector.tensor_copy(den[:], ppv[DH : DH + 1, :])
                        rec = psm.tile([1, 512], F32, tag="rec")
                        nc.vector.reciprocal_approx_fast(rec[:], den[:])
                        recs.append(rec)
                    recBs = []
                    for rec in recs:
                        recB = psm.tile([DH, 512], F32, tag="recB")
                        nc.gpsimd.partition_broadcast(recB[:], rec[:])
                        recBs.append(recB)
                    for hi, h in ((0, hA), (1, hB)):
                        ppv = ppv_A if hi == 0 else ppv_B
                        nc.vector.tensor_tensor(
                            yT[ds(64 * (h % 2), DH), h // 2, ts(tj, 512)],
                            ppv[:DH, :],
                            recBs[hi][:],
                            ALU.mult,
                        )

            def emit_o(tj):
                # o-projection for one 512-row t-block
                for tt in range(4):
                    t0 = 512 * tj + 128 * tt
                    ot = pout.tile([128, C], BF16, tag="o", name="ot")
                    for nb in range(2):
                        po = ps_w.tile([128, 512], F32, tag="w", name="po")
                        for kk in range(2):
                            nc.tensor.matmul(
                                po[:],
                                yT[:, kk, ds(t0, 128)],
                                wo_sb[:, kk, ts(nb, 512)],
                                start=(kk == 0),
                                stop=(kk == 1),
                            )
                        if nb == 0:
                            nc.vector.tensor_copy(ot[:, ts(nb, 512)], po[:])
                        else:
                            nc.scalar.copy(ot[:, ts(nb, 512)], po[:])
                    nc.sync.dma_start(out.ap()[ds(t0, 128), :], ot[:])

            # ---- schedule: interleave per t-block ----------------------
            # warm the PE to full DVFS speed during the initial x-DMA
            # wait with dummy transposes (nothing reads the scratch)
            warm = ps_w.tile([128, 512], BF16, tag="w", name="warm")
            for _ in range(48):
                nc.tensor.transpose(warm[:, 0:128], ident[:], ident[:])
            for tch in range(4):
                emit_transp(tch)
            emit_qk(0)
            for sc in range(4):
                emit_v(sc)
            emit_att(0)
            for tj in range(1, TJ):
                for tch in range(4 * tj, 4 * tj + 4):
                    emit_transp(tch)
                emit_qk(tj)
                emit_o(tj - 1)
                for sc in range(4 * tj, 4 * tj + 4):
                    emit_v(sc)
                emit_att(tj)
            emit_o(TJ - 1)

    nc.compile()
    return nc


_CACHE = {}


def _get_program():
    if "nc" not in _CACHE:
        _CACHE["nc"] = build_program()
    return _CACHE["nc"]


def make_in_maps(x, wq, bq, wk, bk, wv, bv, wo):
    xb16 = np.asarray(x, np.float32).astype(NP_BF16)
    wqb = np.asarray(wq, np.float32).astype(NP_BF16)
    wkb = np.asarray(wk, np.float32).astype(NP_BF16)
    wvb = np.asarray(wv, np.float32).astype(NP_BF16)
    wob = np.asarray(wo, np.float32).astype(NP_BF16)
    in_maps = []
    for core in range(N_CORES):
        b, g = core // 4, core % 4
        sl = slice(g * DQC, (g + 1) * DQC)
        in_maps.append(
            {
                "xb": np.ascontiguousarray(xb16[b]),
                "wq": np.ascontiguousarray(wqb[:, sl]),
                "wk": np.ascontiguousarray(wkb[:, sl]),
                "wv": np.ascontiguousarray(wvb[:, sl]),
                "wo": np.ascontiguousarray(wob[sl, :]),
                "bq": np.ascontiguousarray(np.asarray(bq, np.float32)[sl]),
                "bk": np.ascontiguousarray(np.asarray(bk, np.float32)[sl]),
                "bv": np.ascontiguousarray(np.asarray(bv, np.float32)[sl]),
            }
        )
    return in_maps


def kernel(x, wq, bq, wk, bk, wv, bv, wo, bo):
    from concourse import bass_utils

    nc = _get_program()
    in_maps = make_in_maps(x, wq, bq, wk, bk, wv, bv, wo)
    res = bass_utils.run_bass_kernel_spmd(
        nc, in_maps, core_ids=list(range(N_CORES))
    )
    y = np.zeros((B, T, C), dtype=np.float32)
    for core in range(N_CORES):
        y[core // 4] += res.results[core]["out"].astype(np.float32)
    y += np.asarray(bo, np.float32)
    return y


# revision 21
# speedup vs baseline: 1.5792x; 1.0108x over previous
"""Multi-head masked attention on 8 Trainium2 NeuronCores.

Sharding: data-parallel over batch (B=2 -> 2 groups of 4 cores),
tensor-parallel over heads within a group (16 heads -> 4 heads/core).
Each core computes q/k/v projections for its 4 heads (column-sharded),
causal flash-style attention in the transposed (S^T) domain, and a
row-sharded partial o-projection. The host sums the 4 partials per
batch element and adds the output bias.

All matmul operands are bf16 (inputs converted on host, halving input
DMA); accumulation stays fp32 in PSUM. Output partials are stored
bf16 and summed in fp32 on the host.

The schedule interleaves x-transposes / projections / attention per
512-row t-block so the PE never drains: PV matmuls run one step
behind QK (software pipeline) to hide the exp latency, causal masking
and y-normalization run on the Pool engine, and the reciprocal of the
two heads' softmax denominators is fused into one DVE pass.

Self-contained: hardcodes shapes B=2, T=2048, C=1024, H=16, Dh=64.
"""

import sys

sys.path.insert(0, "/opt/trn_rl_repo")

import numpy as np

import concourse.bass as bass
import concourse.tile as tile
import concourse.mybir as mybir
from concourse import bacc
from concourse.bass import ts, ds
from concourse.masks import make_identity, make_upper_triangular

F32 = mybir.dt.float32
BF16 = mybir.dt.bfloat16
AF = mybir.ActivationFunctionType
ALU = mybir.AluOpType

NP_BF16 = mybir.dt.np(BF16)

B, T, C = 2, 2048, 1024
H, DH = 16, 64
HPC = 4            # heads per core
DQC = HPC * DH     # 256 projected dims per core
N_CORES = 8
NEG = -1.0e30

TC = T // 128    # 16 t-chunks of 128
CC = C // 128    # 8 c-chunks
TJ = T // 512    # 4 t-chunks of 512


def build_program():
    nc = bacc.Bacc("TRN2", target_bir_lowering=False, debug=False)

    xb = nc.dram_tensor("xb", [T, C], BF16, kind="ExternalInput")
    wq = nc.dram_tensor("wq", [C, DQC], BF16, kind="ExternalInput")
    wk = nc.dram_tensor("wk", [C, DQC], BF16, kind="ExternalInput")
    wv = nc.dram_tensor("wv", [C, DQC], BF16, kind="ExternalInput")
    wo = nc.dram_tensor("wo", [DQC, C], BF16, kind="ExternalInput")
    bq = nc.dram_tensor("bq", [DQC], F32, kind="ExternalInput")
    bk = nc.dram_tensor("bk", [DQC], F32, kind="ExternalInput")
    bv = nc.dram_tensor("bv", [DQC], F32, kind="ExternalInput")
    out = nc.dram_tensor("out", [T, C], BF16, kind="ExternalOutput")

    scale = 1.0 / np.sqrt(DH)

    with tile.TileContext(nc) as tc:
        with (
            tc.tile_pool(name="persist", bufs=1) as pp,
            tc.tile_pool(name="ps_s", bufs=2, space="PSUM") as ps_s,
            tc.tile_pool(name="ps_w", bufs=4, space="PSUM") as ps_w,
            tc.tile_pool(name="xin", bufs=16) as px,
            tc.tile_pool(name="xw", bufs=1) as pw,
            tc.tile_pool(name="psb", bufs=3) as pexp,
            tc.tile_pool(name="small", bufs=4) as psm,
            tc.tile_pool(name="outp", bufs=3) as pout,
        ):
            # ---- persistent sbuf tensors -------------------------------
            qT = pp.tile([128, 2, T], BF16, tag="qT")   # [p, pair, t]
            kT = pp.tile([128, 2, T], BF16, tag="kT")
            vA = pp.tile([128, TC, HPC * (DH + 1)], BF16, tag="vA")
            yT = pp.tile([128, 2, T], BF16, tag="yT")
            wo_sb = pp.tile([128, 2, C], BF16, tag="wo")
            ident = pp.tile([128, 128], BF16, tag="ident")
            bqs = pp.tile([128, 2], F32, tag="bqs")
            bks = pp.tile([128, 2], F32, tag="bks")
            bvs = pp.tile([128, DQC], F32, tag="bvs")
            xT = pw.tile([128, CC, T], BF16, tag="xT")
            wq_sb = pw.tile([128, CC, DQC], BF16, tag="wq")
            wk_sb = pw.tile([128, CC, DQC], BF16, tag="wk")
            wv_sb = pw.tile([128, CC, DQC], BF16, tag="wv")

            # constants
            make_identity(nc, ident[:])
            # tri01[s, t] = 1 where t >= s else 0: multiplicative causal
            # mask for the diagonal 128-blocks, applied post-exp on Pool
            tri01 = pp.tile([128, 128], BF16, tag="tri01")
            make_upper_triangular(nc, tri01[:], val=1.0, diag=True)
            # ones column of v_aug (denominator accumulator row)
            vA4 = vA[:].rearrange("p s (h d) -> p s h d", d=DH + 1)
            nc.gpsimd.memset(vA4[:, :, :, DH : DH + 1], 1.0)

            # biases
            nc.sync.dma_start(bqs[:], bq.ap().rearrange("(k p) -> p k", p=128))
            nc.vector.tensor_scalar_mul(bqs[:], bqs[:], scale)
            nc.sync.dma_start(bks[:], bk.ap().rearrange("(k p) -> p k", p=128))
            nc.sync.dma_start(
                bvs[0:1, :], bv.ap().rearrange("(o n) -> o n", o=1)
            )
            nc.gpsimd.partition_broadcast(bvs[:], bvs[0:1, :])

            # ---- input DMAs: x chunks for tj=0 first, then weights -----
            x_tiles = []
            for tch in range(TC):
                x_tile = px.tile([128, C], BF16, tag="x")
                nc.sync.dma_start(x_tile[:], xb.ap()[ts(tch, 128), :])
                x_tiles.append(x_tile)
                if tch == 3:
                    nc.sync.dma_start(
                        wq_sb[:], wq.ap().rearrange("(c p) d -> p c d", p=128)
                    )
                    nc.sync.dma_start(
                        wk_sb[:], wk.ap().rearrange("(c p) d -> p c d", p=128)
                    )
                    nc.sync.dma_start(
                        wv_sb[:], wv.ap().rearrange("(c p) d -> p c d", p=128)
                    )
            nc.sync.dma_start(
                wo_sb[:], wo.ap().rearrange("(k p) n -> p k n", p=128)
            )

            # ---- emission helpers --------------------------------------
            def emit_transp(tch):
                # x^T for one 128-row chunk via PE transposes
                for ccg in range(CC // 4):
                    pt = ps_w.tile([128, 512], BF16, tag="w", name="pt")
                    for q in range(4):
                        cc = 4 * ccg + q
                        nc.tensor.transpose(
                            pt[:, ts(q, 128)],
                            x_tiles[tch][:, ts(cc, 128)],
                            ident[:],
                        )
                    dst = xT[:, ds(4 * ccg, 4), ts(tch, 128)]
                    src = pt[:].rearrange("p (c t) -> p c t", t=128)
                    # alternate DVE/Act explicitly; never Pool (no PSUM
                    # access) and keep its ucode library stable
                    if ccg == 0:
                        nc.vector.tensor_copy(dst, src)
                    else:
                        nc.scalar.copy(dst, src)

            def emit_qk(tj):
                # Q^T / K^T projections for one 512-col t-block, both pairs
                for hp in range(2):
                    pq = ps_w.tile([128, 512], F32, tag="w", name="pq")
                    for cc in range(CC):
                        nc.tensor.matmul(
                            pq[:],
                            wq_sb[:, cc, ts(hp, 128)],
                            xT[:, cc, ts(tj, 512)],
                            start=(cc == 0),
                            stop=(cc == CC - 1),
                        )
                    nc.vector.tensor_scalar(
                        qT[:, hp, ts(tj, 512)],
                        pq[:],
                        scale,
                        bqs[:, hp : hp + 1],
                        ALU.mult,
                        ALU.add,
                    )
                    pk = ps_w.tile([128, 512], F32, tag="w", name="pk")
                    for cc in range(CC):
                        nc.tensor.matmul(
                            pk[:],
                            wk_sb[:, cc, ts(hp, 128)],
                            xT[:, cc, ts(tj, 512)],
                            start=(cc == 0),
                            stop=(cc == CC - 1),
                        )
                    nc.vector.tensor_scalar(
                        kT[:, hp, ts(tj, 512)],
                        pk[:],
                        bks[:, hp : hp + 1],
                        None,
                        ALU.add,
                    )

            def emit_v(sc):
                # V projection (natural layout) + bias for one 128-row chunk
                pv = ps_w.tile([128, 512], F32, tag="w", name="pv")
                for cc in range(CC):
                    nc.tensor.matmul(
                        pv[:, :DQC],
                        xT[:, cc, ts(sc, 128)],
                        wv_sb[:, cc, :],
                        start=(cc == 0),
                        stop=(cc == CC - 1),
                    )
                nc.vector.tensor_tensor(
                    vA4[:, sc, :, :DH],
                    pv[:, :DQC].rearrange("p (h d) -> p h d", d=DH),
                    bvs[:].rearrange("p (h d) -> p h d", d=DH),
                    ALU.add,
                )

            def emit_att(tj):
                n_sc = 4 * (tj + 1)
                ppvs = {}

                def emit_pv(item):
                    hp, sc, off, psb = item
                    ppv_A, ppv_B = ppvs[hp]
                    for hi, h in ((0, 2 * hp), (1, 2 * hp + 1)):
                        ppv = ppv_A if hi == 0 else ppv_B
                        nc.tensor.matmul(
                            ppv[: DH + 1, ds(off, 512 - off)],
                            vA[:, sc, ds(h * (DH + 1), DH + 1)],
                            psb[:, ds(512 * hi + off, 512 - off)],
                            start=(sc == 0),
                            stop=(sc == n_sc - 1),
                        )

                def normalize(hp):
                    # normalize: y^T = Y_unnorm^T * (1/denom); the fast
                    # single-pass reciprocal (~18 bits) is far inside
                    # the bf16 error budget. Emission order keeps DVE
                    # from idling on the Pool broadcast round-trip.
                    ppv_A, ppv_B = ppvs[hp]
                    recs = []
                    for hi in (0, 1):
                        ppv = ppv_A if hi == 0 else ppv_B
                        den = psm.tile([1, 512], F32, tag="den")
                        nc.vector.tensor_copy(den[:], ppv[DH : DH + 1, :])
                        rec = psm.tile([1, 512], F32, tag="rec")
                        nc.vector.reciprocal_approx_fast(rec[:], den[:])
                        recs.append(rec)
                    recBs = []
                    for rec in recs:
                        recB = psm.tile([DH, 512], F32, tag="recB")
                        nc.gpsimd.partition_broadcast(recB[:], rec[:])
                        recBs.append(recB)
                    for hi, h in ((0, 2 * hp), (1, 2 * hp + 1)):
                        ppv = ppv_A if hi == 0 else ppv_B
                        nc.vector.tensor_tensor(
                            yT[ds(64 * (h % 2), DH), h // 2, ts(tj, 512)],
                            ppv[:DH, :],
                            recBs[hi][:],
                            ALU.mult,
                        )

                # the PV software pipeline is carried ACROSS the two
                # head-pair passes so the PE never drains at the seam
                prev = None
                for hp in range(2):
                    for sc in range(n_sc):
                        if sc == 0:
                            ppvs[hp] = (
                                ps_w.tile([128, 512], F32, tag="w",
                                          name="ppv_A"),
                                ps_w.tile([128, 512], F32, tag="w",
                                          name="ppv_B"),
                            )
                        kd = sc - 4 * tj  # >=0 on the causal diagonal
                        off = 128 * kd if kd > 0 else 0
                        # tj=0 is Act-overhead-bound: keep QK full width
                        # there so one full-width exp suffices (the extra
                        # columns are garbage-but-unread; PV trims at off)
                        qlo = off if tj > 0 else 0
                        pss = ps_s.tile([128, 1024], F32, tag="s", name="pss")
                        # QK^T for both heads of the pair, row-packed.
                        # Columns [0, qlo) are fully masked -> skipped.
                        for hi, (half, ppos) in enumerate(
                            [(0, (0, 0)), (512, (64, 0))]
                        ):
                            prow = slice(64 * hi, 64 * hi + 64)
                            nc.tensor.matmul(
                                pss[:, ds(half + qlo, 512 - qlo)],
                                kT[prow, hp, ts(sc, 128)],
                                qT[prow, hp, ds(512 * tj + qlo, 512 - qlo)],
                                start=True,
                                stop=True,
                                tile_position=ppos,
                            )
                        psb = pexp.tile([128, 1024], BF16, tag="p", name="psb")
                        if qlo == 0:
                            nc.scalar.activation(psb[:], pss[:], AF.Exp)
                        else:
                            # only the written windows (cols [0,qlo) of
                            # each half were skipped by the QK matmul)
                            for half in (0, 512):
                                nc.scalar.activation(
                                    psb[:, ds(half + qlo, 512 - qlo)],
                                    pss[:, ds(half + qlo, 512 - qlo)],
                                    AF.Exp,
                                )
                        if kd >= 0:
                            # causal mask on the diagonal 128-block of
                            # each head: zero p where s > t (post-exp,
                            # cheap bf16 multiply on DVE; Pool only runs
                            # partition_broadcast to avoid ucode-library
                            # swaps that head-of-line block its queue)
                            for half in (0, 512):
                                blk = psb[:, ds(half + off, 128)]
                                nc.vector.tensor_tensor(
                                    blk, blk, tri01[:], ALU.mult
                                )
                        # PV runs one step behind QK so the PE is not
                        # blocked on the exp of the current step
                        if prev is not None:
                            emit_pv(prev)
                            if prev[0] != hp:
                                normalize(prev[0])
                        prev = (hp, sc, off, psb)
                emit_pv(prev)
                normalize(1)

            def emit_o(tj):
                # o-projection for one 512-row t-block; all kk=0 matmuls
                # of a group issue before the kk=1 ones so the PE is not
                # blocked on the second pair's normalize
                for ttg in range(2):
                    tts = (2 * ttg, 2 * ttg + 1)
                    ots = {
                        tt: pout.tile([128, C], BF16, tag="o", name="ot")
                        for tt in tts
                    }
                    chains = []
                    for tt in tts:
                        t0 = 512 * tj + 128 * tt
                        for nb in range(2):
                            po = ps_w.tile([128, 512], F32, tag="w",
                                           name="po")
                            nc.tensor.matmul(
                                po[:],
                                yT[:, 0, ds(t0, 128)],
                                wo_sb[:, 0, ts(nb, 512)],
                                start=True,
                                stop=False,
                            )
                            chains.append((tt, t0, nb, po))
                    for tt, t0, nb, po in chains:
                        nc.tensor.matmul(
                            po[:],
                            yT[:, 1, ds(t0, 128)],
                            wo_sb[:, 1, ts(nb, 512)],
                            start=False,
                            stop=True,
                        )
                        if nb == 0:
                            nc.vector.tensor_copy(ots[tt][:, ts(nb, 512)],
                                                  po[:])
                        else:
                            nc.scalar.copy(ots[tt][:, ts(nb, 512)], po[:])
                    for tt in tts:
                        t0 = 512 * tj + 128 * tt
                        nc.sync.dma_start(out.ap()[ds(t0, 128), :],
                                          ots[tt][:])

            # ---- schedule: interleave per t-block ----------------------
            # warm the PE to full DVFS speed during the initial x-DMA
            # wait with dummy transposes (nothing reads the scratch)
            warm = ps_w.tile([128, 512], BF16, tag="w", name="warm")
            for _ in range(48):
                nc.tensor.transpose(warm[:, 0:128], ident[:], ident[:])
            for tch in range(4):
                emit_transp(tch)
            emit_qk(0)
            for sc in range(4):
                emit_v(sc)
            emit_att(0)
            for tj in range(1, TJ):
                for tch in range(4 * tj, 4 * tj + 4):
                    emit_transp(tch)
                emit_qk(tj)
                emit_o(tj - 1)
                for sc in range(4 * tj, 4 * tj + 4):
                    emit_v(sc)
                emit_att(tj)
            emit_o(TJ - 1)

    nc.compile()
    return nc


_CACHE = {}


def _get_program():
    if "nc" not in _CACHE:
        _CACHE["nc"] = build_program()
    return _CACHE["nc"]


def make_in_maps(x, wq, bq, wk, bk, wv, bv, wo):
    xb16 = np.asarray(x, np.float32).astype(NP_BF16)
    wqb = np.asarray(wq, np.float32).astype(NP_BF16)
    wkb = np.asarray(wk, np.float32).astype(NP_BF16)
    wvb = np.asarray(wv, np.float32).astype(NP_BF16)
    wob = np.asarray(wo, np.float32).astype(NP_BF16)
    in_maps = []
    for core in range(N_CORES):
        b, g = core // 4, core % 4
        sl = slice(g * DQC, (g + 1) * DQC)
        in_maps.append(
            {
                "xb": np.ascontiguousarray(xb16[b]),
                "wq": np.ascontiguousarray(wqb[:, sl]),
                "wk": np.ascontiguousarray(wkb[:, sl]),
                "wv": np.ascontiguousarray(wvb[:, sl]),
                "wo": np.ascontiguousarray(wob[sl, :]),
                "bq": np.ascontiguousarray(np.asarray(bq, np.float32)[sl]),
                "bk": np.ascontiguousarray(np.asarray(bk, np.float32)[sl]),
                "bv": np.ascontiguousarray(np.asarray(bv, np.float32)[sl]),
            }
        )
    return in_maps


def kernel(x, wq, bq, wk, bk, wv, bv, wo, bo):
    from concourse import bass_utils

    nc = _get_program()
    in_maps = make_in_maps(x, wq, bq, wk, bk, wv, bv, wo)
    res = bass_utils.run_bass_kernel_spmd(
        nc, in_maps, core_ids=list(range(N_CORES))
    )
    y = np.zeros((B, T, C), dtype=np.float32)
    for core in range(N_CORES):
        y[core // 4] += res.results[core]["out"].astype(np.float32)
    y += np.asarray(bo, np.float32)
    return y


# revision 25
# speedup vs baseline: 1.5836x; 1.0028x over previous
"""Multi-head masked attention on 8 Trainium2 NeuronCores.

Sharding: data-parallel over batch (B=2 -> 2 groups of 4 cores),
tensor-parallel over heads within a group (16 heads -> 4 heads/core).
Each core computes q/k/v projections for its 4 heads (column-sharded),
causal flash-style attention in the transposed (S^T) domain, and a
row-sharded partial o-projection. The host sums the 4 partials per
batch element and adds the output bias.

All matmul operands are bf16 (inputs converted on host, halving input
DMA); accumulation stays fp32 in PSUM. Output partials are stored
bf16 and summed in fp32 on the host.

The schedule interleaves x-transposes / projections / attention per
512-row t-block so the PE never drains: PV matmuls run one step
behind QK (software pipeline) to hide the exp latency, causal masking
and y-normalization run on the Pool engine, and the reciprocal of the
two heads' softmax denominators is fused into one DVE pass.

Self-contained: hardcodes shapes B=2, T=2048, C=1024, H=16, Dh=64.
"""

import sys

sys.path.insert(0, "/opt/trn_rl_repo")

import numpy as np

import concourse.bass as bass
import concourse.tile as tile
import concourse.mybir as mybir
from concourse import bacc
from concourse.bass import ts, ds
from concourse.masks import make_identity, make_upper_triangular

F32 = mybir.dt.float32
BF16 = mybir.dt.bfloat16
AF = mybir.ActivationFunctionType
ALU = mybir.AluOpType

NP_BF16 = mybir.dt.np(BF16)

B, T, C = 2, 2048, 1024
H, DH = 16, 64
HPC = 4            # heads per core
DQC = HPC * DH     # 256 projected dims per core
N_CORES = 8
NEG = -1.0e30

TC = T // 128    # 16 t-chunks of 128
CC = C // 128    # 8 c-chunks
TJ = T // 512    # 4 t-chunks of 512


def build_program():
    nc = bacc.Bacc("TRN2", target_bir_lowering=False, debug=False)

    xb = nc.dram_tensor("xb", [T, C], BF16, kind="ExternalInput")
    wq = nc.dram_tensor("wq", [C, DQC], BF16, kind="ExternalInput")
    wk = nc.dram_tensor("wk", [C, DQC], BF16, kind="ExternalInput")
    wv = nc.dram_tensor("wv", [C, DQC], BF16, kind="ExternalInput")
    wo = nc.dram_tensor("wo", [DQC, C], BF16, kind="ExternalInput")
    bq = nc.dram_tensor("bq", [DQC], F32, kind="ExternalInput")
    bk = nc.dram_tensor("bk", [DQC], F32, kind="ExternalInput")
    bv = nc.dram_tensor("bv", [DQC], F32, kind="ExternalInput")
    out = nc.dram_tensor("out", [T, C], BF16, kind="ExternalOutput")

    scale = 1.0 / np.sqrt(DH)

    with tile.TileContext(nc) as tc:
        with (
            tc.tile_pool(name="persist", bufs=1) as pp,
            tc.tile_pool(name="ps_s", bufs=2, space="PSUM") as ps_s,
            tc.tile_pool(name="ps_w", bufs=4, space="PSUM") as ps_w,
            tc.tile_pool(name="xin", bufs=16) as px,
            tc.tile_pool(name="xw", bufs=1) as pw,
            tc.tile_pool(name="psb", bufs=3) as pexp,
            tc.tile_pool(name="small", bufs=4) as psm,
            tc.tile_pool(name="outp", bufs=3) as pout,
        ):
            # ---- persistent sbuf tensors -------------------------------
            qT = pp.tile([128, 2, T], BF16, tag="qT")   # [p, pair, t]
            kT = pp.tile([128, 2, T], BF16, tag="kT")
            vA = pp.tile([128, TC, HPC * (DH + 1)], BF16, tag="vA")
            yT = pp.tile([128, 2, T], BF16, tag="yT")
            wo_sb = pp.tile([128, 2, C], BF16, tag="wo")
            ident = pp.tile([128, 128], BF16, tag="ident")
            bqs = pp.tile([128, 2], F32, tag="bqs")
            bks = pp.tile([128, 2], F32, tag="bks")
            bvs = pp.tile([128, DQC], F32, tag="bvs")
            xT = pw.tile([128, CC, T], BF16, tag="xT")
            wq_sb = pw.tile([128, CC, DQC], BF16, tag="wq")
            wk_sb = pw.tile([128, CC, DQC], BF16, tag="wk")
            wv_sb = pw.tile([128, CC, DQC], BF16, tag="wv")

            # constants
            make_identity(nc, ident[:])
            # tri01[s, t] = 1 where t >= s else 0: multiplicative causal
            # mask for the diagonal 128-blocks, applied post-exp on Pool
            tri01 = pp.tile([128, 128], BF16, tag="tri01")
            make_upper_triangular(nc, tri01[:], val=1.0, diag=True)
            # ones column of v_aug (denominator accumulator row)
            vA4 = vA[:].rearrange("p s (h d) -> p s h d", d=DH + 1)
            nc.gpsimd.memset(vA4[:, :, :, DH : DH + 1], 1.0)

            # biases
            nc.sync.dma_start(bqs[:], bq.ap().rearrange("(k p) -> p k", p=128))
            nc.vector.tensor_scalar_mul(bqs[:], bqs[:], scale)
            nc.sync.dma_start(bks[:], bk.ap().rearrange("(k p) -> p k", p=128))
            nc.sync.dma_start(
                bvs[0:1, :], bv.ap().rearrange("(o n) -> o n", o=1)
            )
            nc.gpsimd.partition_broadcast(bvs[:], bvs[0:1, :])

            # ---- input DMAs: x chunks for tj=0 first, then weights -----
            x_tiles = []
            for tch in range(TC):
                x_tile = px.tile([128, C], BF16, tag="x")
                nc.sync.dma_start(x_tile[:], xb.ap()[ts(tch, 128), :])
                x_tiles.append(x_tile)
                if tch == 3:
                    nc.sync.dma_start(
                        wq_sb[:], wq.ap().rearrange("(c p) d -> p c d", p=128)
                    )
                    nc.sync.dma_start(
                        wk_sb[:], wk.ap().rearrange("(c p) d -> p c d", p=128)
                    )
                    nc.sync.dma_start(
                        wv_sb[:], wv.ap().rearrange("(c p) d -> p c d", p=128)
                    )
            nc.sync.dma_start(
                wo_sb[:], wo.ap().rearrange("(k p) n -> p k n", p=128)
            )

            # ---- emission helpers --------------------------------------
            def emit_transp(tch):
                # x^T for one 128-row chunk via PE transposes
                for ccg in range(CC // 4):
                    pt = ps_w.tile([128, 512], BF16, tag="w", name="pt")
                    for q in range(4):
                        cc = 4 * ccg + q
                        nc.tensor.transpose(
                            pt[:, ts(q, 128)],
                            x_tiles[tch][:, ts(cc, 128)],
                            ident[:],
                        )
                    dst = xT[:, ds(4 * ccg, 4), ts(tch, 128)]
                    src = pt[:].rearrange("p (c t) -> p c t", t=128)
                    # alternate DVE/Act explicitly; never Pool (no PSUM
                    # access) and keep its ucode library stable
                    if ccg == 0:
                        nc.vector.tensor_copy(dst, src)
                    else:
                        nc.scalar.copy(dst, src)

            def emit_qk(tj):
                # Q^T / K^T projections for one 512-col t-block, both pairs
                for hp in range(2):
                    pq = ps_w.tile([128, 512], F32, tag="w", name="pq")
                    for cc in range(CC):
                        nc.tensor.matmul(
                            pq[:],
                            wq_sb[:, cc, ts(hp, 128)],
                            xT[:, cc, ts(tj, 512)],
                            start=(cc == 0),
                            stop=(cc == CC - 1),
                        )
                    # scale+bias on Act (Copy shares the exp table, so no
                    # act-table reload) to keep DVE clear for attention
                    nc.scalar.activation(
                        qT[:, hp, ts(tj, 512)],
                        pq[:],
                        AF.Identity,
                        bias=bqs[:, hp : hp + 1],
                        scale=scale,
                    )
                    pk = ps_w.tile([128, 512], F32, tag="w", name="pk")
                    for cc in range(CC):
                        nc.tensor.matmul(
                            pk[:],
                            wk_sb[:, cc, ts(hp, 128)],
                            xT[:, cc, ts(tj, 512)],
                            start=(cc == 0),
                            stop=(cc == CC - 1),
                        )
                    nc.scalar.activation(
                        kT[:, hp, ts(tj, 512)],
                        pk[:],
                        AF.Identity,
                        bias=bks[:, hp : hp + 1],
                    )

            def emit_v(sc):
                # V projection (natural layout) + bias for one 128-row chunk
                pv = ps_w.tile([128, 512], F32, tag="w", name="pv")
                for cc in range(CC):
                    nc.tensor.matmul(
                        pv[:, :DQC],
                        xT[:, cc, ts(sc, 128)],
                        wv_sb[:, cc, :],
                        start=(cc == 0),
                        stop=(cc == CC - 1),
                    )
                nc.vector.tensor_tensor(
                    vA4[:, sc, :, :DH],
                    pv[:, :DQC].rearrange("p (h d) -> p h d", d=DH),
                    bvs[:].rearrange("p (h d) -> p h d", d=DH),
                    ALU.add,
                )

            def emit_att(tj):
                n_sc = 4 * (tj + 1)
                ppvs = {}

                def emit_pv(item):
                    hp, sc, off, psb = item
                    ppv_A, ppv_B = ppvs[hp]
                    for hi, h in ((0, 2 * hp), (1, 2 * hp + 1)):
                        ppv = ppv_A if hi == 0 else ppv_B
                        nc.tensor.matmul(
                            ppv[: DH + 1, ds(off, 512 - off)],
                            vA[:, sc, ds(h * (DH + 1), DH + 1)],
                            psb[:, ds(512 * hi + off, 512 - off)],
                            start=(sc == 0),
                            stop=(sc == n_sc - 1),
                        )

                def normalize(hp):
                    # normalize: y^T = Y_unnorm^T * (1/denom); the fast
                    # single-pass reciprocal (~18 bits) is far inside
                    # the bf16 error budget. Emission order keeps DVE
                    # from idling on the Pool broadcast round-trip.
                    ppv_A, ppv_B = ppvs[hp]
                    recs = []
                    for hi in (0, 1):
                        ppv = ppv_A if hi == 0 else ppv_B
                        den = psm.tile([1, 512], F32, tag="den")
                        nc.vector.tensor_copy(den[:], ppv[DH : DH + 1, :])
                        rec = psm.tile([1, 512], F32, tag="rec")
                        nc.vector.reciprocal_approx_fast(rec[:], den[:])
                        recs.append(rec)
                    recBs = []
                    for rec in recs:
                        recB = psm.tile([DH, 512], F32, tag="recB")
                        nc.gpsimd.partition_broadcast(recB[:], rec[:])
                        recBs.append(recB)
                    for hi, h in ((0, 2 * hp), (1, 2 * hp + 1)):
                        ppv = ppv_A if hi == 0 else ppv_B
                        nc.vector.tensor_tensor(
                            yT[ds(64 * (h % 2), DH), h // 2, ts(tj, 512)],
                            ppv[:DH, :],
                            recBs[hi][:],
                            ALU.mult,
                        )

                # the PV software pipeline is carried ACROSS the two
                # head-pair passes so the PE never drains at the seam
                prev = None
                for hp in range(2):
                    for sc in range(n_sc):
                        if sc == 0:
                            ppvs[hp] = (
                                ps_w.tile([128, 512], F32, tag="w",
                                          name="ppv_A"),
                                ps_w.tile([128, 512], F32, tag="w",
                                          name="ppv_B"),
                            )
                        kd = sc - 4 * tj  # >=0 on the causal diagonal
                        off = 128 * kd if kd > 0 else 0
                        # tj=0 is Act-overhead-bound: keep QK full width
                        # there so one full-width exp suffices (the extra
                        # columns are garbage-but-unread; PV trims at off)
                        qlo = off if tj > 0 else 0
                        pss = ps_s.tile([128, 1024], F32, tag="s", name="pss")
                        # QK^T for both heads of the pair, row-packed.
                        # Columns [0, qlo) are fully masked -> skipped.
                        for hi, (half, ppos) in enumerate(
                            [(0, (0, 0)), (512, (64, 0))]
                        ):
                            prow = slice(64 * hi, 64 * hi + 64)
                            nc.tensor.matmul(
                                pss[:, ds(half + qlo, 512 - qlo)],
                                kT[prow, hp, ts(sc, 128)],
                                qT[prow, hp, ds(512 * tj + qlo, 512 - qlo)],
                                start=True,
                                stop=True,
                                tile_position=ppos,
                            )
                        psb = pexp.tile([128, 1024], BF16, tag="p", name="psb")
                        if qlo == 0:
                            nc.scalar.activation(psb[:], pss[:], AF.Exp)
                        else:
                            # only the written windows (cols [0,qlo) of
                            # each half were skipped by the QK matmul)
                            for half in (0, 512):
                                nc.scalar.activation(
                                    psb[:, ds(half + qlo, 512 - qlo)],
                                    pss[:, ds(half + qlo, 512 - qlo)],
                                    AF.Exp,
                                )
                        if kd >= 0:
                            # causal mask on the diagonal 128-block of
                            # each head: zero p where s > t (post-exp,
                            # cheap bf16 multiply on DVE; Pool only runs
                            # partition_broadcast to avoid ucode-library
                            # swaps that head-of-line block its queue)
                            for half in (0, 512):
                                blk = psb[:, ds(half + off, 128)]
                                nc.vector.tensor_tensor(
                                    blk, blk, tri01[:], ALU.mult
                                )
                        # PV runs one step behind QK so the PE is not
                        # blocked on the exp of the current step
                        if prev is not None:
                            emit_pv(prev)
                            if prev[0] != hp:
                                normalize(prev[0])
                        prev = (hp, sc, off, psb)
                emit_pv(prev)
                normalize(1)

            def emit_o(tj):
                # o-projection for one 512-row t-block; all kk=0 matmuls
                # of a group issue before the kk=1 ones so the PE is not
                # blocked on the second pair's normalize
                for ttg in range(2):
                    tts = (2 * ttg, 2 * ttg + 1)
                    ots = {
                        tt: pout.tile([128, C], BF16, tag="o", name="ot")
                        for tt in tts
                    }
                    chains = []
                    for tt in tts:
                        t0 = 512 * tj + 128 * tt
                        for nb in range(2):
                            po = ps_w.tile([128, 512], F32, tag="w",
                                           name="po")
                            nc.tensor.matmul(
                                po[:],
                                yT[:, 0, ds(t0, 128)],
                                wo_sb[:, 0, ts(nb, 512)],
                                start=True,
                                stop=False,
                            )
                            chains.append((tt, t0, nb, po))
                    for tt, t0, nb, po in chains:
                        nc.tensor.matmul(
                            po[:],
                            yT[:, 1, ds(t0, 128)],
                            wo_sb[:, 1, ts(nb, 512)],
                            start=False,
                            stop=True,
                        )
                        if nb == 0:
                            nc.vector.tensor_copy(ots[tt][:, ts(nb, 512)],
                                                  po[:])
                        else:
                            nc.scalar.copy(ots[tt][:, ts(nb, 512)], po[:])
                    for tt in tts:
                        t0 = 512 * tj + 128 * tt
                        nc.sync.dma_start(out.ap()[ds(t0, 128), :],
                                          ots[tt][:])

            # ---- schedule: interleave per t-block ----------------------
            # warm the PE to full DVFS speed during the initial x-DMA
            # wait with dummy transposes (nothing reads the scratch)
            warm = ps_w.tile([128, 512], BF16, tag="w", name="warm")
            for _ in range(60):
                nc.tensor.transpose(warm[:, 0:128], ident[:], ident[:])
            for tch in range(4):
                emit_transp(tch)
            emit_qk(0)
            for sc in range(4):
                emit_v(sc)
            emit_att(0)
            for tj in range(1, TJ):
                for tch in range(4 * tj, 4 * tj + 4):
                    emit_transp(tch)
                emit_qk(tj)
                emit_o(tj - 1)
                for sc in range(4 * tj, 4 * tj + 4):
                    emit_v(sc)
                emit_att(tj)
            emit_o(TJ - 1)

    nc.compile()
    return nc


_CACHE = {}


def _get_program():
    if "nc" not in _CACHE:
        _CACHE["nc"] = build_program()
    return _CACHE["nc"]


def make_in_maps(x, wq, bq, wk, bk, wv, bv, wo):
    xb16 = np.asarray(x, np.float32).astype(NP_BF16)
    wqb = np.asarray(wq, np.float32).astype(NP_BF16)
    wkb = np.asarray(wk, np.float32).astype(NP_BF16)
    wvb = np.asarray(wv, np.float32).astype(NP_BF16)
    wob = np.asarray(wo, np.float32).astype(NP_BF16)
    in_maps = []
    for core in range(N_CORES):
        b, g = core // 4, core % 4
        sl = slice(g * DQC, (g + 1) * DQC)
        in_maps.append(
            {
                "xb": np.ascontiguousarray(xb16[b]),
                "wq": np.ascontiguousarray(wqb[:, sl]),
                "wk": np.ascontiguousarray(wkb[:, sl]),
                "wv": np.ascontiguousarray(wvb[:, sl]),
                "wo": np.ascontiguousarray(wob[sl, :]),
                "bq": np.ascontiguousarray(np.asarray(bq, np.float32)[sl]),
                "bk": np.ascontiguousarray(np.asarray(bk, np.float32)[sl]),
                "bv": np.ascontiguousarray(np.asarray(bv, np.float32)[sl]),
            }
        )
    return in_maps


def kernel(x, wq, bq, wk, bk, wv, bv, wo, bo):
    from concourse import bass_utils

    nc = _get_program()
    in_maps = make_in_maps(x, wq, bq, wk, bk, wv, bv, wo)
    res = bass_utils.run_bass_kernel_spmd(
        nc, in_maps, core_ids=list(range(N_CORES))
    )
    y = np.zeros((B, T, C), dtype=np.float32)
    for core in range(N_CORES):
        y[core // 4] += res.results[core]["out"].astype(np.float32)
    y += np.asarray(bo, np.float32)
    return y


# revision 26
# speedup vs baseline: 1.6666x; 1.0524x over previous
"""Multi-head masked attention on 8 Trainium2 NeuronCores.

Sharding: data-parallel over batch (B=2 -> 2 groups of 4 cores),
tensor-parallel over heads within a group (16 heads -> 4 heads/core).
Each core computes q/k/v projections for its 4 heads (column-sharded),
causal flash-style attention in the transposed (S^T) domain, and a
row-sharded partial o-projection. The host sums the 4 partials per
batch element and adds the output bias.

All matmul operands are bf16 (inputs converted on host, halving input
DMA); accumulation stays fp32 in PSUM. Output partials are stored
bf16 and summed in fp32 on the host.

The schedule interleaves x-transposes / projections / attention per
512-row t-block so the PE never drains: PV matmuls run one step
behind QK (software pipeline) to hide the exp latency, causal masking
and y-normalization run on the Pool engine, and the reciprocal of the
two heads' softmax denominators is fused into one DVE pass.

Self-contained: hardcodes shapes B=2, T=2048, C=1024, H=16, Dh=64.
"""

import sys

sys.path.insert(0, "/opt/trn_rl_repo")

import numpy as np

import concourse.bass as bass
import concourse.tile as tile
import concourse.mybir as mybir
from concourse import bacc
from concourse.bass import ts, ds
from concourse.masks import make_identity, make_upper_triangular

F32 = mybir.dt.float32
BF16 = mybir.dt.bfloat16
AF = mybir.ActivationFunctionType
ALU = mybir.AluOpType

NP_BF16 = mybir.dt.np(BF16)

B, T, C = 2, 2048, 1024
H, DH = 16, 64
HPC = 4            # heads per core
DQC = HPC * DH     # 256 projected dims per core
N_CORES = 8
NEG = -1.0e30

TC = T // 128    # 16 t-chunks of 128
CC = C // 128    # 8 c-chunks
TJ = T // 512    # 4 t-chunks of 512


def build_program():
    nc = bacc.Bacc("TRN2", target_bir_lowering=False, debug=False)

    xb = nc.dram_tensor("xb", [T, C], BF16, kind="ExternalInput")
    wq = nc.dram_tensor("wq", [C, DQC], BF16, kind="ExternalInput")
    wk = nc.dram_tensor("wk", [C, DQC], BF16, kind="ExternalInput")
    wv = nc.dram_tensor("wv", [C, DQC], BF16, kind="ExternalInput")
    wo = nc.dram_tensor("wo", [DQC, C], BF16, kind="ExternalInput")
    bq = nc.dram_tensor("bq", [DQC], F32, kind="ExternalInput")
    bk = nc.dram_tensor("bk", [DQC], F32, kind="ExternalInput")
    bv = nc.dram_tensor("bv", [DQC], F32, kind="ExternalInput")
    out = nc.dram_tensor("out", [T, C], BF16, kind="ExternalOutput")

    scale = 1.0 / np.sqrt(DH)

    with tile.TileContext(nc) as tc:
        with (
            tc.tile_pool(name="persist", bufs=1) as pp,
            tc.tile_pool(name="ps_s", bufs=2, space="PSUM") as ps_s,
            tc.tile_pool(name="ps_w", bufs=4, space="PSUM") as ps_w,
            tc.tile_pool(name="xw", bufs=1) as pw,
            tc.tile_pool(name="psb", bufs=3) as pexp,
            tc.tile_pool(name="small", bufs=4) as psm,
            tc.tile_pool(name="outp", bufs=3) as pout,
        ):
            # ---- persistent sbuf tensors -------------------------------
            qT = pp.tile([128, 2, T], BF16, tag="qT")   # [p, pair, t]
            kT = pp.tile([128, 2, T], BF16, tag="kT")
            vA = pp.tile([128, TC, HPC * (DH + 1)], BF16, tag="vA")
            yT = pp.tile([128, 2, T], BF16, tag="yT")
            wo_sb = pp.tile([128, 2, C], BF16, tag="wo")
            ident = pp.tile([128, 128], BF16, tag="ident")
            bqs = pp.tile([128, 2], F32, tag="bqs")
            bks = pp.tile([128, 2], F32, tag="bks")
            bvs = pp.tile([128, DQC], F32, tag="bvs")
            xT = pw.tile([128, CC, T], BF16, tag="xT")
            wq_sb = pw.tile([128, CC, DQC], BF16, tag="wq")
            wk_sb = pw.tile([128, CC, DQC], BF16, tag="wk")
            wv_sb = pw.tile([128, CC, DQC], BF16, tag="wv")

            # constants
            make_identity(nc, ident[:])
            # tri01[s, t] = 1 where t >= s else 0: multiplicative causal
            # mask for the diagonal 128-blocks, applied post-exp on Pool
            tri01 = pp.tile([128, 128], BF16, tag="tri01")
            make_upper_triangular(nc, tri01[:], val=1.0, diag=True)
            # ones column of v_aug (denominator accumulator row)
            vA4 = vA[:].rearrange("p s (h d) -> p s h d", d=DH + 1)
            nc.gpsimd.memset(vA4[:, :, :, DH : DH + 1], 1.0)

            # biases
            nc.sync.dma_start(bqs[:], bq.ap().rearrange("(k p) -> p k", p=128))
            nc.vector.tensor_scalar_mul(bqs[:], bqs[:], scale)
            nc.sync.dma_start(bks[:], bk.ap().rearrange("(k p) -> p k", p=128))
            nc.sync.dma_start(
                bvs[0:1, :], bv.ap().rearrange("(o n) -> o n", o=1)
            )
            nc.gpsimd.partition_broadcast(bvs[:], bvs[0:1, :])

            # ---- input DMAs: x^T comes straight from DRAM via the DMA
            # xbar transpose (2-byte dtype), one quarter per t-block so
            # projections chase the transfer; no PE transposes at all
            for tjq in range(TJ):
                nc.sync.dma_start_transpose(
                    xT[:, :, ts(tjq, 512)], xb.ap()[ts(tjq, 512), :]
                )
                if tjq == 0:
                    nc.sync.dma_start(
                        wq_sb[:], wq.ap().rearrange("(c p) d -> p c d", p=128)
                    )
                    nc.sync.dma_start(
                        wk_sb[:], wk.ap().rearrange("(c p) d -> p c d", p=128)
                    )
                    nc.sync.dma_start(
                        wv_sb[:], wv.ap().rearrange("(c p) d -> p c d", p=128)
                    )
            nc.sync.dma_start(
                wo_sb[:], wo.ap().rearrange("(k p) n -> p k n", p=128)
            )

            # ---- emission helpers --------------------------------------
            def emit_qk(tj):
                # Q^T / K^T projections for one 512-col t-block, both pairs
                for hp in range(2):
                    pq = ps_w.tile([128, 512], F32, tag="w", name="pq")
                    for cc in range(CC):
                        nc.tensor.matmul(
                            pq[:],
                            wq_sb[:, cc, ts(hp, 128)],
                            xT[:, cc, ts(tj, 512)],
                            start=(cc == 0),
                            stop=(cc == CC - 1),
                        )
                    # scale+bias on Act (Copy shares the exp table, so no
                    # act-table reload) to keep DVE clear for attention
                    nc.scalar.activation(
                        qT[:, hp, ts(tj, 512)],
                        pq[:],
                        AF.Identity,
                        bias=bqs[:, hp : hp + 1],
                        scale=scale,
                    )
                    pk = ps_w.tile([128, 512], F32, tag="w", name="pk")
                    for cc in range(CC):
                        nc.tensor.matmul(
                            pk[:],
                            wk_sb[:, cc, ts(hp, 128)],
                            xT[:, cc, ts(tj, 512)],
                            start=(cc == 0),
                            stop=(cc == CC - 1),
                        )
                    nc.scalar.activation(
                        kT[:, hp, ts(tj, 512)],
                        pk[:],
                        AF.Identity,
                        bias=bks[:, hp : hp + 1],
                    )

            def emit_v(sc):
                # V projection (natural layout) + bias for one 128-row chunk
                pv = ps_w.tile([128, 512], F32, tag="w", name="pv")
                for cc in range(CC):
                    nc.tensor.matmul(
                        pv[:, :DQC],
                        xT[:, cc, ts(sc, 128)],
                        wv_sb[:, cc, :],
                        start=(cc == 0),
                        stop=(cc == CC - 1),
                    )
                nc.vector.tensor_tensor(
                    vA4[:, sc, :, :DH],
                    pv[:, :DQC].rearrange("p (h d) -> p h d", d=DH),
                    bvs[:].rearrange("p (h d) -> p h d", d=DH),
                    ALU.add,
                )

            def emit_att(tj):
                n_sc = 4 * (tj + 1)
                ppvs = {}

                def emit_pv(item):
                    hp, sc, off, psb = item
                    ppv_A, ppv_B = ppvs[hp]
                    for hi, h in ((0, 2 * hp), (1, 2 * hp + 1)):
                        ppv = ppv_A if hi == 0 else ppv_B
                        nc.tensor.matmul(
                            ppv[: DH + 1, ds(off, 512 - off)],
                            vA[:, sc, ds(h * (DH + 1), DH + 1)],
                            psb[:, ds(512 * hi + off, 512 - off)],
                            start=(sc == 0),
                            stop=(sc == n_sc - 1),
                        )

                def normalize(hp):
                    # normalize: y^T = Y_unnorm^T * (1/denom); the fast
                    # single-pass reciprocal (~18 bits) is far inside
                    # the bf16 error budget. Emission order keeps DVE
                    # from idling on the Pool broadcast round-trip.
                    ppv_A, ppv_B = ppvs[hp]
                    recs = []
                    for hi in (0, 1):
                        ppv = ppv_A if hi == 0 else ppv_B
                        den = psm.tile([1, 512], F32, tag="den")
                        nc.vector.tensor_copy(den[:], ppv[DH : DH + 1, :])
                        rec = psm.tile([1, 512], F32, tag="rec")
                        nc.vector.reciprocal_approx_fast(rec[:], den[:])
                        recs.append(rec)
                    recBs = []
                    for rec in recs:
                        recB = psm.tile([DH, 512], F32, tag="recB")
                        nc.gpsimd.partition_broadcast(recB[:], rec[:])
                        recBs.append(recB)
                    for hi, h in ((0, 2 * hp), (1, 2 * hp + 1)):
                        ppv = ppv_A if hi == 0 else ppv_B
                        nc.vector.tensor_tensor(
                            yT[ds(64 * (h % 2), DH), h // 2, ts(tj, 512)],
                            ppv[:DH, :],
                            recBs[hi][:],
                            ALU.mult,
                        )

                # the PV software pipeline is carried ACROSS the two
                # head-pair passes so the PE never drains at the seam
                prev = None
                for hp in range(2):
                    for sc in range(n_sc):
                        if sc == 0:
                            ppvs[hp] = (
                                ps_w.tile([128, 512], F32, tag="w",
                                          name="ppv_A"),
                                ps_w.tile([128, 512], F32, tag="w",
                                          name="ppv_B"),
                            )
                        kd = sc - 4 * tj  # >=0 on the causal diagonal
                        off = 128 * kd if kd > 0 else 0
                        # tj=0 is Act-overhead-bound: keep QK full width
                        # there so one full-width exp suffices (the extra
                        # columns are garbage-but-unread; PV trims at off)
                        qlo = off if tj > 0 else 0
                        pss = ps_s.tile([128, 1024], F32, tag="s", name="pss")
                        # QK^T for both heads of the pair, row-packed.
                        # Columns [0, qlo) are fully masked -> skipped.
                        for hi, (half, ppos) in enumerate(
                            [(0, (0, 0)), (512, (64, 0))]
                        ):
                            prow = slice(64 * hi, 64 * hi + 64)
                            nc.tensor.matmul(
                                pss[:, ds(half + qlo, 512 - qlo)],
                                kT[prow, hp, ts(sc, 128)],
                                qT[prow, hp, ds(512 * tj + qlo, 512 - qlo)],
                                start=True,
                                stop=True,
                                tile_position=ppos,
                            )
                        psb = pexp.tile([128, 1024], BF16, tag="p", name="psb")
                        if qlo == 0:
                            nc.scalar.activation(psb[:], pss[:], AF.Exp)
                        else:
                            # only the written windows (cols [0,qlo) of
                            # each half were skipped by the QK matmul)
                            for half in (0, 512):
                                nc.scalar.activation(
                                    psb[:, ds(half + qlo, 512 - qlo)],
                                    pss[:, ds(half + qlo, 512 - qlo)],
                                    AF.Exp,
                                )
                        if kd >= 0:
                            # causal mask on the diagonal 128-block of
                            # each head: zero p where s > t (post-exp,
                            # cheap bf16 multiply on DVE; Pool only runs
                            # partition_broadcast to avoid ucode-library
                            # swaps that head-of-line block its queue)
                            for half in (0, 512):
                                blk = psb[:, ds(half + off, 128)]
                                nc.vector.tensor_tensor(
                                    blk, blk, tri01[:], ALU.mult
                                )
                        # PV runs one step behind QK so the PE is not
                        # blocked on the exp of the current step
                        if prev is not None:
                            emit_pv(prev)
                            if prev[0] != hp:
                                normalize(prev[0])
                        prev = (hp, sc, off, psb)
                emit_pv(prev)
                normalize(1)

            def emit_o(tj):
                # o-projection for one 512-row t-block; all kk=0 matmuls
                # of a group issue before the kk=1 ones so the PE is not
                # blocked on the second pair's normalize
                for ttg in range(2):
                    tts = (2 * ttg, 2 * ttg + 1)
                    ots = {
                        tt: pout.tile([128, C], BF16, tag="o", name="ot")
                        for tt in tts
                    }
                    chains = []
                    for tt in tts:
                        t0 = 512 * tj + 128 * tt
                        for nb in range(2):
                            po = ps_w.tile([128, 512], F32, tag="w",
                                           name="po")
                            nc.tensor.matmul(
                                po[:],
                                yT[:, 0, ds(t0, 128)],
                                wo_sb[:, 0, ts(nb, 512)],
                                start=True,
                                stop=False,
                            )
                            chains.append((tt, t0, nb, po))
                    for tt, t0, nb, po in chains:
                        nc.tensor.matmul(
                            po[:],
                            yT[:, 1, ds(t0, 128)],
                            wo_sb[:, 1, ts(nb, 512)],
                            start=False,
                            stop=True,
                        )
                        if nb == 0:
                            nc.vector.tensor_copy(ots[tt][:, ts(nb, 512)],
                                                  po[:])
                        else:
                            nc.scalar.copy(ots[tt][:, ts(nb, 512)], po[:])
                    for tt in tts:
                        t0 = 512 * tj + 128 * tt
                        nc.sync.dma_start(out.ap()[ds(t0, 128), :],
                                          ots[tt][:])

            # ---- schedule: interleave per t-block ----------------------
            # warm the PE to full DVFS speed during the initial x-DMA
            # wait with dummy transposes (nothing reads the scratch)
            warm = ps_w.tile([128, 512], BF16, tag="w", name="warm")
            for _ in range(80):
                nc.tensor.transpose(warm[:, 0:128], ident[:], ident[:])
            emit_qk(0)
            for sc in range(4):
                emit_v(sc)
            emit_att(0)
            for tj in range(1, TJ):
                emit_qk(tj)
                emit_o(tj - 1)
                for sc in range(4 * tj, 4 * tj + 4):
                    emit_v(sc)
                emit_att(tj)
            emit_o(TJ - 1)

    nc.compile()
    return nc


_CACHE = {}


def _get_program():
    if "nc" not in _CACHE:
        _CACHE["nc"] = build_program()
    return _CACHE["nc"]


def make_in_maps(x, wq, bq, wk, bk, wv, bv, wo):
    xb16 = np.asarray(x, np.float32).astype(NP_BF16)
    wqb = np.asarray(wq, np.float32).astype(NP_BF16)
    wkb = np.asarray(wk, np.float32).astype(NP_BF16)
    wvb = np.asarray(wv, np.float32).astype(NP_BF16)
    wob = np.asarray(wo, np.float32).astype(NP_BF16)
    in_maps = []
    for core in range(N_CORES):
        b, g = core // 4, core % 4
        sl = slice(g * DQC, (g + 1) * DQC)
        in_maps.append(
            {
                "xb": np.ascontiguousarray(xb16[b]),
                "wq": np.ascontiguousarray(wqb[:, sl]),
                "wk": np.ascontiguousarray(wkb[:, sl]),
                "wv": np.ascontiguousarray(wvb[:, sl]),
                "wo": np.ascontiguousarray(wob[sl, :]),
                "bq": np.ascontiguousarray(np.asarray(bq, np.float32)[sl]),
                "bk": np.ascontiguousarray(np.asarray(bk, np.float32)[sl]),
                "bv": np.ascontiguousarray(np.asarray(bv, np.float32)[sl]),
            }
        )
    return in_maps


def kernel(x, wq, bq, wk, bk, wv, bv, wo, bo):
    from concourse import bass_utils

    nc = _get_program()
    in_maps = make_in_maps(x, wq, bq, wk, bk, wv, bv, wo)
    res = bass_utils.run_bass_kernel_spmd(
        nc, in_maps, core_ids=list(range(N_CORES))
    )
    y = np.zeros((B, T, C), dtype=np.float32)
    for core in range(N_CORES):
        y[core // 4] += res.results[core]["out"].astype(np.float32)
    y += np.asarray(bo, np.float32)
    return y


# revision 28
# speedup vs baseline: 1.6855x; 1.0114x over previous
"""Multi-head masked attention on 8 Trainium2 NeuronCores.

Sharding: data-parallel over batch (B=2 -> 2 groups of 4 cores),
tensor-parallel over heads within a group (16 heads -> 4 heads/core).
Each core computes q/k/v projections for its 4 heads (column-sharded),
causal flash-style attention in the transposed (S^T) domain, and a
row-sharded partial o-projection. The host sums the 4 partials per
batch element and adds the output bias.

All matmul operands are bf16 (inputs converted on host, halving input
DMA); accumulation stays fp32 in PSUM. Output partials are stored
bf16 and summed in fp32 on the host.

x^T is produced straight from DRAM by the DMA xbar transpose (no PE
transposes). The schedule interleaves projections / attention /
o-projection per 512-row t-block so the PE never drains: PV matmuls
run one step behind QK (software pipeline, carried across head-pair
seams) to hide the exp latency, causal masking is a post-exp
multiplicative triangular mask on DVE, y-normalization uses the
single-pass DVE reciprocal with the broadcast on Pool, and Q/K
bias+scale run on the Act engine (Identity, same act table as Exp).

Self-contained: hardcodes shapes B=2, T=2048, C=1024, H=16, Dh=64.
"""

import sys

sys.path.insert(0, "/opt/trn_rl_repo")

import numpy as np

import concourse.bass as bass
import concourse.tile as tile
import concourse.mybir as mybir
from concourse import bacc
from concourse.bass import ts, ds
from concourse.masks import make_identity, make_upper_triangular

F32 = mybir.dt.float32
BF16 = mybir.dt.bfloat16
AF = mybir.ActivationFunctionType
ALU = mybir.AluOpType

NP_BF16 = mybir.dt.np(BF16)

B, T, C = 2, 2048, 1024
H, DH = 16, 64
HPC = 4            # heads per core
DQC = HPC * DH     # 256 projected dims per core
N_CORES = 8
NEG = -1.0e30

TC = T // 128    # 16 t-chunks of 128
CC = C // 128    # 8 c-chunks
TJ = T // 512    # 4 t-chunks of 512


def build_program():
    nc = bacc.Bacc("TRN2", target_bir_lowering=False, debug=False)

    xb = nc.dram_tensor("xb", [T, C], BF16, kind="ExternalInput")
    wq = nc.dram_tensor("wq", [C, DQC], BF16, kind="ExternalInput")
    wk = nc.dram_tensor("wk", [C, DQC], BF16, kind="ExternalInput")
    wv = nc.dram_tensor("wv", [C, DQC], BF16, kind="ExternalInput")
    wo = nc.dram_tensor("wo", [DQC, C], BF16, kind="ExternalInput")
    bq = nc.dram_tensor("bq", [DQC], F32, kind="ExternalInput")
    bk = nc.dram_tensor("bk", [DQC], F32, kind="ExternalInput")
    bv = nc.dram_tensor("bv", [DQC], F32, kind="ExternalInput")
    out = nc.dram_tensor("out", [T, C], BF16, kind="ExternalOutput")

    scale = 1.0 / np.sqrt(DH)

    with tile.TileContext(nc) as tc:
        with (
            tc.tile_pool(name="persist", bufs=1) as pp,
            tc.tile_pool(name="ps_s", bufs=2, space="PSUM") as ps_s,
            tc.tile_pool(name="ps_w", bufs=4, space="PSUM") as ps_w,
            tc.tile_pool(name="xw", bufs=1) as pw,
            tc.tile_pool(name="psb", bufs=3) as pexp,
            tc.tile_pool(name="small", bufs=4) as psm,
            tc.tile_pool(name="outp", bufs=3) as pout,
        ):
            # ---- persistent sbuf tensors -------------------------------
            qT = pp.tile([128, 2, T], BF16, tag="qT")   # [p, pair, t]
            kT = pp.tile([128, 2, T], BF16, tag="kT")
            vA = pp.tile([128, TC, HPC * (DH + 1)], BF16, tag="vA")
            yT = pp.tile([128, 2, T], BF16, tag="yT")
            wo_sb = pp.tile([128, 2, C], BF16, tag="wo")
            ident = pp.tile([128, 128], BF16, tag="ident")
            bqs = pp.tile([128, 2], F32, tag="bqs")
            bks = pp.tile([128, 2], F32, tag="bks")
            bvs = pp.tile([128, DQC], F32, tag="bvs")
            xT = pw.tile([128, CC, T], BF16, tag="xT")
            wq_sb = pw.tile([128, CC, DQC], BF16, tag="wq")
            wk_sb = pw.tile([128, CC, DQC], BF16, tag="wk")
            wv_sb = pw.tile([128, CC, DQC], BF16, tag="wv")

            # constants
            make_identity(nc, ident[:])
            # tri01[s, t] = 1 where t >= s else 0: multiplicative causal
            # mask for the diagonal 128-blocks, applied post-exp on Pool
            tri01 = pp.tile([128, 128], BF16, tag="tri01")
            make_upper_triangular(nc, tri01[:], val=1.0, diag=True)
            # ones column of v_aug (denominator accumulator row)
            vA4 = vA[:].rearrange("p s (h d) -> p s h d", d=DH + 1)
            nc.gpsimd.memset(vA4[:, :, :, DH : DH + 1], 1.0)

            # biases
            nc.sync.dma_start(bqs[:], bq.ap().rearrange("(k p) -> p k", p=128))
            nc.vector.tensor_scalar_mul(bqs[:], bqs[:], scale)
            nc.sync.dma_start(bks[:], bk.ap().rearrange("(k p) -> p k", p=128))
            nc.sync.dma_start(
                bvs[0:1, :], bv.ap().rearrange("(o n) -> o n", o=1)
            )
            nc.gpsimd.partition_broadcast(bvs[:], bvs[0:1, :])

            # ---- input DMAs: x^T comes straight from DRAM via the DMA
            # xbar transpose (2-byte dtype), one quarter per t-block so
            # projections chase the transfer; no PE transposes at all
            for tjq in range(TJ):
                nc.sync.dma_start_transpose(
                    xT[:, :, ts(tjq, 512)], xb.ap()[ts(tjq, 512), :]
                )
                if tjq == 0:
                    nc.sync.dma_start(
                        wq_sb[:], wq.ap().rearrange("(c p) d -> p c d", p=128)
                    )
                    nc.sync.dma_start(
                        wk_sb[:], wk.ap().rearrange("(c p) d -> p c d", p=128)
                    )
                    nc.sync.dma_start(
                        wv_sb[:], wv.ap().rearrange("(c p) d -> p c d", p=128)
                    )
            nc.sync.dma_start(
                wo_sb[:], wo.ap().rearrange("(k p) n -> p k n", p=128)
            )

            # ---- emission helpers --------------------------------------
            def emit_qk(tj):
                # Q^T / K^T projections for one 512-col t-block, both pairs
                for hp in range(2):
                    pq = ps_w.tile([128, 512], F32, tag="w", name="pq")
                    for cc in range(CC):
                        nc.tensor.matmul(
                            pq[:],
                            wq_sb[:, cc, ts(hp, 128)],
                            xT[:, cc, ts(tj, 512)],
                            start=(cc == 0),
                            stop=(cc == CC - 1),
                        )
                    # scale+bias on Act (Copy shares the exp table, so no
                    # act-table reload) to keep DVE clear for attention
                    nc.scalar.activation(
                        qT[:, hp, ts(tj, 512)],
                        pq[:],
                        AF.Identity,
                        bias=bqs[:, hp : hp + 1],
                        scale=scale,
                    )
                    pk = ps_w.tile([128, 512], F32, tag="w", name="pk")
                    for cc in range(CC):
                        nc.tensor.matmul(
                            pk[:],
                            wk_sb[:, cc, ts(hp, 128)],
                            xT[:, cc, ts(tj, 512)],
                            start=(cc == 0),
                            stop=(cc == CC - 1),
                        )
                    nc.scalar.activation(
                        kT[:, hp, ts(tj, 512)],
                        pk[:],
                        AF.Identity,
                        bias=bks[:, hp : hp + 1],
                    )

            def emit_v(sc):
                # V projection (natural layout) + bias for one 128-row chunk
                pv = ps_w.tile([128, 512], F32, tag="w", name="pv")
                for cc in range(CC):
                    nc.tensor.matmul(
                        pv[:, :DQC],
                        xT[:, cc, ts(sc, 128)],
                        wv_sb[:, cc, :],
                        start=(cc == 0),
                        stop=(cc == CC - 1),
                    )
                nc.vector.tensor_tensor(
                    vA4[:, sc, :, :DH],
                    pv[:, :DQC].rearrange("p (h d) -> p h d", d=DH),
                    bvs[:].rearrange("p (h d) -> p h d", d=DH),
                    ALU.add,
                )

            def emit_att(tj):
                n_sc = 4 * (tj + 1)
                ppvs = {}

                def emit_pv(item):
                    hp, sc, off, psb = item
                    ppv_A, ppv_B = ppvs[hp]
                    for hi, h in ((0, 2 * hp), (1, 2 * hp + 1)):
                        ppv = ppv_A if hi == 0 else ppv_B
                        nc.tensor.matmul(
                            ppv[: DH + 1, ds(off, 512 - off)],
                            vA[:, sc, ds(h * (DH + 1), DH + 1)],
                            psb[:, ds(512 * hi + off, 512 - off)],
                            start=(sc == 0),
                            stop=(sc == n_sc - 1),
                        )

                def normalize(hp):
                    # normalize: y^T = Y_unnorm^T * (1/denom); the fast
                    # single-pass reciprocal (~18 bits) is far inside
                    # the bf16 error budget. Emission order keeps DVE
                    # from idling on the Pool broadcast round-trip.
                    ppv_A, ppv_B = ppvs[hp]
                    recs = []
                    for hi in (0, 1):
                        ppv = ppv_A if hi == 0 else ppv_B
                        den = psm.tile([1, 512], F32, tag="den")
                        nc.vector.tensor_copy(den[:], ppv[DH : DH + 1, :])
                        rec = psm.tile([1, 512], F32, tag="rec")
                        nc.vector.reciprocal_approx_fast(rec[:], den[:])
                        recs.append(rec)
                    recBs = []
                    for rec in recs:
                        recB = psm.tile([DH, 512], F32, tag="recB")
                        nc.gpsimd.partition_broadcast(recB[:], rec[:])
                        recBs.append(recB)
                    for hi, h in ((0, 2 * hp), (1, 2 * hp + 1)):
                        ppv = ppv_A if hi == 0 else ppv_B
                        nc.vector.tensor_tensor(
                            yT[ds(64 * (h % 2), DH), h // 2, ts(tj, 512)],
                            ppv[:DH, :],
                            recBs[hi][:],
                            ALU.mult,
                        )

                # the PV software pipeline is carried ACROSS the two
                # head-pair passes so the PE never drains at the seam
                prev = None
                for hp in range(2):
                    for sc in range(n_sc):
                        if sc == 0:
                            ppvs[hp] = (
                                ps_w.tile([128, 512], F32, tag="w",
                                          name="ppv_A"),
                                ps_w.tile([128, 512], F32, tag="w",
                                          name="ppv_B"),
                            )
                        kd = sc - 4 * tj  # >=0 on the causal diagonal
                        off = 128 * kd if kd > 0 else 0
                        # tj=0 is Act-overhead-bound: keep QK full width
                        # there so one full-width exp suffices (the extra
                        # columns are garbage-but-unread; PV trims at off)
                        qlo = off if tj > 0 else 0
                        pss = ps_s.tile([128, 1024], F32, tag="s", name="pss")
                        # QK^T for both heads of the pair, row-packed.
                        # Columns [0, qlo) are fully masked -> skipped.
                        for hi, (half, ppos) in enumerate(
                            [(0, (0, 0)), (512, (64, 0))]
                        ):
                            prow = slice(64 * hi, 64 * hi + 64)
                            nc.tensor.matmul(
                                pss[:, ds(half + qlo, 512 - qlo)],
                                kT[prow, hp, ts(sc, 128)],
                                qT[prow, hp, ds(512 * tj + qlo, 512 - qlo)],
                                start=True,
                                stop=True,
                                tile_position=ppos,
                            )
                        psb = pexp.tile([128, 1024], BF16, tag="p", name="psb")
                        if qlo == 0:
                            nc.scalar.activation(psb[:], pss[:], AF.Exp)
                        else:
                            # only the written windows (cols [0,qlo) of
                            # each half were skipped by the QK matmul)
                            for half in (0, 512):
                                nc.scalar.activation(
                                    psb[:, ds(half + qlo, 512 - qlo)],
                                    pss[:, ds(half + qlo, 512 - qlo)],
                                    AF.Exp,
                                )
                        if kd >= 0:
                            # causal mask on the diagonal 128-block of
                            # each head: zero p where s > t (post-exp,
                            # cheap bf16 multiply on DVE; Pool only runs
                            # partition_broadcast to avoid ucode-library
                            # swaps that head-of-line block its queue)
                            for half in (0, 512):
                                blk = psb[:, ds(half + off, 128)]
                                nc.vector.tensor_tensor(
                                    blk, blk, tri01[:], ALU.mult
                                )
                        # PV runs one step behind QK so the PE is not
                        # blocked on the exp of the current step
                        if prev is not None:
                            emit_pv(prev)
                            if prev[0] != hp:
                                normalize(prev[0])
                        prev = (hp, sc, off, psb)
                emit_pv(prev)
                normalize(1)

            def emit_o(tj):
                # o-projection for one 512-row t-block; all kk=0 matmuls
                # of a group issue before the kk=1 ones so the PE is not
                # blocked on the second pair's normalize
                for ttg in range(2):
                    tts = (2 * ttg, 2 * ttg + 1)
                    ots = {
                        tt: pout.tile([128, C], BF16, tag="o", name="ot")
                        for tt in tts
                    }
                    chains = []
                    for tt in tts:
                        t0 = 512 * tj + 128 * tt
                        for nb in range(2):
                            po = ps_s.tile([128, 512], F32, tag="s",
                                           name="po")
                            nc.tensor.matmul(
                                po[:],
                                yT[:, 0, ds(t0, 128)],
                                wo_sb[:, 0, ts(nb, 512)],
                                start=True,
                                stop=False,
                            )
                            chains.append((tt, t0, nb, po))
                    for tt, t0, nb, po in chains:
                        nc.tensor.matmul(
                            po[:],
                            yT[:, 1, ds(t0, 128)],
                            wo_sb[:, 1, ts(nb, 512)],
                            start=False,
                            stop=True,
                        )
                        if nb == 0:
                            nc.vector.tensor_copy(ots[tt][:, ts(nb, 512)],
                                                  po[:])
                        else:
                            nc.scalar.copy(ots[tt][:, ts(nb, 512)], po[:])
                    for tt in tts:
                        t0 = 512 * tj + 128 * tt
                        nc.sync.dma_start(out.ap()[ds(t0, 128), :],
                                          ots[tt][:])

            # ---- schedule: interleave per t-block ----------------------
            # warm the PE to full DVFS speed during the initial x-DMA
            # wait with dummy transposes (nothing reads the scratch)
            warm = ps_w.tile([128, 512], BF16, tag="w", name="warm")
            for _ in range(150):
                nc.tensor.transpose(warm[:, 0:128], ident[:], ident[:])
            emit_qk(0)
            for sc in range(4):
                emit_v(sc)
            emit_att(0)
            for tj in range(1, TJ):
                emit_qk(tj)
                emit_o(tj - 1)
                for sc in range(4 * tj, 4 * tj + 4):
                    emit_v(sc)
                emit_att(tj)
            emit_o(TJ - 1)

    nc.compile()
    return nc


_CACHE = {}


def _get_program():
    if "nc" not in _CACHE:
        _CACHE["nc"] = build_program()
    return _CACHE["nc"]


def make_in_maps(x, wq, bq, wk, bk, wv, bv, wo):
    xb16 = np.asarray(x, np.float32).astype(NP_BF16)
    wqb = np.asarray(wq, np.float32).astype(NP_BF16)
    wkb = np.asarray(wk, np.float32).astype(NP_BF16)
    wvb = np.asarray(wv, np.float32).astype(NP_BF16)
    wob = np.asarray(wo, np.float32).astype(NP_BF16)
    in_maps = []
    for core in range(N_CORES):
        b, g = core // 4, core % 4
        sl = slice(g * DQC, (g + 1) * DQC)
        in_maps.append(
            {
                "xb": np.ascontiguousarray(xb16[b]),
                "wq": np.ascontiguousarray(wqb[:, sl]),
                "wk": np.ascontiguousarray(wkb[:, sl]),
                "wv": np.ascontiguousarray(wvb[:, sl]),
                "wo": np.ascontiguousarray(wob[sl, :]),
                "bq": np.ascontiguousarray(np.asarray(bq, np.float32)[sl]),
                "bk": np.ascontiguousarray(np.asarray(bk, np.float32)[sl]),
                "bv": np.ascontiguousarray(np.asarray(bv, np.float32)[sl]),
            }
        )
    return in_maps


def kernel(x, wq, bq, wk, bk, wv, bv, wo, bo):
    from concourse import bass_utils

    nc = _get_program()
    in_maps = make_in_maps(x, wq, bq, wk, bk, wv, bv, wo)
    res = bass_utils.run_bass_kernel_spmd(
        nc, in_maps, core_ids=list(range(N_CORES))
    )
    y = np.zeros((B, T, C), dtype=np.float32)
    for core in range(N_CORES):
        y[core // 4] += res.results[core]["out"].astype(np.float32)
    y += np.asarray(bo, np.float32)
    return y


# revision 34
# speedup vs baseline: 1.7163x; 1.0183x over previous
"""Multi-head masked attention on 8 Trainium2 NeuronCores.

Sharding: data-parallel over batch (B=2 -> 2 groups of 4 cores),
tensor-parallel over heads within a group (16 heads -> 4 heads/core).
Each core computes q/k/v projections for its 4 heads (column-sharded),
causal flash-style attention in the transposed (S^T) domain, and a
row-sharded partial o-projection. The host sums the 4 partials per
batch element and adds the output bias.

All matmul operands are bf16 (inputs converted on host, halving input
DMA); accumulation stays fp32 in PSUM. Output partials are stored
bf16 and summed in fp32 on the host.

x^T is produced straight from DRAM by the DMA xbar transpose (no PE
transposes). The schedule interleaves projections / attention /
o-projection per 512-row t-block so the PE never drains: PV matmuls
run one step behind QK (software pipeline, carried across head-pair
seams) to hide the exp latency, causal masking is a post-exp
multiplicative triangular mask on DVE, y-normalization uses the
single-pass DVE reciprocal with the broadcast on Pool, and Q/K
bias+scale run on the Act engine (Identity, same act table as Exp).

Self-contained: hardcodes shapes B=2, T=2048, C=1024, H=16, Dh=64.
"""

import sys

sys.path.insert(0, "/opt/trn_rl_repo")

import numpy as np

import concourse.bass as bass
import concourse.tile as tile
import concourse.mybir as mybir
from concourse import bacc
from concourse.bass import ts, ds
from concourse.masks import make_identity, make_upper_triangular

F32 = mybir.dt.float32
BF16 = mybir.dt.bfloat16
AF = mybir.ActivationFunctionType
ALU = mybir.AluOpType

NP_BF16 = mybir.dt.np(BF16)

B, T, C = 2, 2048, 1024
H, DH = 16, 64
HPC = 4            # heads per core
DQC = HPC * DH     # 256 projected dims per core
N_CORES = 8
NEG = -1.0e30

TC = T // 128    # 16 t-chunks of 128
CC = C // 128    # 8 c-chunks
TJ = T // 512    # 4 t-chunks of 512


def build_program():
    nc = bacc.Bacc("TRN2", target_bir_lowering=False, debug=False)

    xb = nc.dram_tensor("xb", [T, C], BF16, kind="ExternalInput")
    wq = nc.dram_tensor("wq", [C, DQC], BF16, kind="ExternalInput")
    wk = nc.dram_tensor("wk", [C, DQC], BF16, kind="ExternalInput")
    wv = nc.dram_tensor("wv", [C, DQC], BF16, kind="ExternalInput")
    wo = nc.dram_tensor("wo", [DQC, C], BF16, kind="ExternalInput")
    bq = nc.dram_tensor("bq", [DQC], F32, kind="ExternalInput")
    bk = nc.dram_tensor("bk", [DQC], F32, kind="ExternalInput")
    bv = nc.dram_tensor("bv", [DQC], F32, kind="ExternalInput")
    out = nc.dram_tensor("out", [T, C], BF16, kind="ExternalOutput")

    scale = 1.0 / np.sqrt(DH)

    with tile.TileContext(nc) as tc:
        with (
            tc.tile_pool(name="persist", bufs=1) as pp,
            tc.tile_pool(name="ps_s", bufs=2, space="PSUM") as ps_s,
            tc.tile_pool(name="ps_w", bufs=4, space="PSUM") as ps_w,
            tc.tile_pool(name="xw", bufs=1) as pw,
            tc.tile_pool(name="psb", bufs=3) as pexp,
            tc.tile_pool(name="small", bufs=4) as psm,
            tc.tile_pool(name="outp", bufs=3) as pout,
        ):
            # ---- persistent sbuf tensors -------------------------------
            qT = pp.tile([128, 2, T], BF16, tag="qT")   # [p, pair, t]
            kT = pp.tile([128, 2, T], BF16, tag="kT")
            vA = pp.tile([128, TC, HPC * (DH + 1)], BF16, tag="vA")
            yT = pp.tile([128, 2, T], BF16, tag="yT")
            wo_sb = pp.tile([128, 2, C], BF16, tag="wo")
            ident = pp.tile([128, 128], BF16, tag="ident")
            bqs = pp.tile([128, 2], F32, tag="bqs")
            bks = pp.tile([128, 2], F32, tag="bks")
            bvs = pp.tile([128, DQC], F32, tag="bvs")
            xT = pw.tile([128, CC, T], BF16, tag="xT")
            wq_sb = pw.tile([128, CC, DQC], BF16, tag="wq")
            wk_sb = pw.tile([128, CC, DQC], BF16, tag="wk")
            wv_sb = pw.tile([128, CC, DQC], BF16, tag="wv")

            # constants
            make_identity(nc, ident[:])
            # tri01[s, t] = 1 where t >= s else 0: multiplicative causal
            # mask for the diagonal 128-blocks, applied post-exp on Pool
            tri01 = pp.tile([128, 128], BF16, tag="tri01")
            make_upper_triangular(nc, tri01[:], val=1.0, diag=True)
            # ones column of v_aug (denominator accumulator row)
            vA4 = vA[:].rearrange("p s (h d) -> p s h d", d=DH + 1)
            nc.gpsimd.memset(vA4[:, :, :, DH : DH + 1], 1.0)

            # biases
            nc.sync.dma_start(bqs[:], bq.ap().rearrange("(k p) -> p k", p=128))
            nc.vector.tensor_scalar_mul(bqs[:], bqs[:], scale)
            nc.sync.dma_start(bks[:], bk.ap().rearrange("(k p) -> p k", p=128))
            nc.sync.dma_start(
                bvs[0:1, :], bv.ap().rearrange("(o n) -> o n", o=1)
            )
            nc.gpsimd.partition_broadcast(bvs[:], bvs[0:1, :])

            # ---- input DMAs: x^T comes straight from DRAM via the DMA
            # xbar transpose (2-byte dtype), one quarter per t-block so
            # projections chase the transfer; no PE transposes at all
            for tjq in range(TJ):
                nc.sync.dma_start_transpose(
                    xT[:, :, ts(tjq, 512)], xb.ap()[ts(tjq, 512), :]
                )
                if tjq == 0:
                    nc.sync.dma_start(
                        wq_sb[:], wq.ap().rearrange("(c p) d -> p c d", p=128)
                    )
                    nc.sync.dma_start(
                        wk_sb[:], wk.ap().rearrange("(c p) d -> p c d", p=128)
                    )
                    nc.sync.dma_start(
                        wv_sb[:], wv.ap().rearrange("(c p) d -> p c d", p=128)
                    )
            nc.sync.dma_start(
                wo_sb[:], wo.ap().rearrange("(k p) n -> p k n", p=128)
            )

            # ---- emission helpers --------------------------------------
            def emit_qk(tj):
                # Q^T / K^T projections for one 512-col t-block, both pairs
                for hp in range(2):
                    pq = ps_w.tile([128, 512], F32, tag="w", name="pq")
                    for cc in range(CC):
                        nc.tensor.matmul(
                            pq[:],
                            wq_sb[:, cc, ts(hp, 128)],
                            xT[:, cc, ts(tj, 512)],
                            start=(cc == 0),
                            stop=(cc == CC - 1),
                        )
                    # scale+bias on Act (Copy shares the exp table, so no
                    # act-table reload) to keep DVE clear for attention
                    nc.scalar.activation(
                        qT[:, hp, ts(tj, 512)],
                        pq[:],
                        AF.Identity,
                        bias=bqs[:, hp : hp + 1],
                        scale=scale,
                    )
                    pk = ps_w.tile([128, 512], F32, tag="w", name="pk")
                    for cc in range(CC):
                        nc.tensor.matmul(
                            pk[:],
                            wk_sb[:, cc, ts(hp, 128)],
                            xT[:, cc, ts(tj, 512)],
                            start=(cc == 0),
                            stop=(cc == CC - 1),
                        )
                    nc.scalar.activation(
                        kT[:, hp, ts(tj, 512)],
                        pk[:],
                        AF.Identity,
                        bias=bks[:, hp : hp + 1],
                    )

            def emit_v(sc):
                # V projection (natural layout) + bias for one 128-row chunk
                pv = ps_w.tile([128, 512], F32, tag="w", name="pv")
                for cc in range(CC):
                    nc.tensor.matmul(
                        pv[:, :DQC],
                        xT[:, cc, ts(sc, 128)],
                        wv_sb[:, cc, :],
                        start=(cc == 0),
                        stop=(cc == CC - 1),
                    )
                nc.vector.tensor_tensor(
                    vA4[:, sc, :, :DH],
                    pv[:, :DQC].rearrange("p (h d) -> p h d", d=DH),
                    bvs[:].rearrange("p (h d) -> p h d", d=DH),
                    ALU.add,
                )

            def emit_att_pre(tj, steps):
                # emit QK+exp+mask for the first `steps` sc of head-pair 0
                # (no PV, no PSUM accumulators) to prefill the pipeline
                items = []
                for sc in range(steps):
                    items.append(emit_qkexp(tj, 0, sc))
                return items

            def emit_qkexp(tj, hp, sc):
                n_sc = 4 * (tj + 1)
                kd = sc - 4 * tj
                off = 128 * kd if kd > 0 else 0
                qlo = off if tj > 0 else 0
                pss = ps_s.tile([128, 1024], F32, tag="s", name="pss")
                for hi, (half, ppos) in enumerate(
                    [(0, (0, 0)), (512, (64, 0))]
                ):
                    prow = slice(64 * hi, 64 * hi + 64)
                    nc.tensor.matmul(
                        pss[:, ds(half + qlo, 512 - qlo)],
                        kT[prow, hp, ts(sc, 128)],
                        qT[prow, hp, ds(512 * tj + qlo, 512 - qlo)],
                        start=True,
                        stop=True,
                        tile_position=ppos,
                    )
                psb = pexp.tile([128, 1024], BF16, tag="p", name="psb")
                if qlo == 0:
                    nc.scalar.activation(psb[:], pss[:], AF.Exp)
                else:
                    for half in (0, 512):
                        nc.scalar.activation(
                            psb[:, ds(half + qlo, 512 - qlo)],
                            pss[:, ds(half + qlo, 512 - qlo)],
                            AF.Exp,
                        )
                if kd >= 0:
                    for half in (0, 512):
                        blk = psb[:, ds(half + off, 128)]
                        nc.vector.tensor_tensor(blk, blk, tri01[:], ALU.mult)
                return (hp, sc, off, psb)

            def emit_att(tj, pre=()):
                n_sc = 4 * (tj + 1)
                ppvs = {}
                sc0 = {0: len(pre), 1: 0}

                def alloc_ppv():
                    return (
                        ps_w.tile([128, 512], F32, tag="w", name="ppv_A"),
                        ps_w.tile([128, 512], F32, tag="w", name="ppv_B"),
                    )

                def emit_pv(item):
                    hp, sc, off, psb = item
                    ppv_A, ppv_B = ppvs[hp]
                    for hi, h in ((0, 2 * hp), (1, 2 * hp + 1)):
                        ppv = ppv_A if hi == 0 else ppv_B
                        nc.tensor.matmul(
                            ppv[: DH + 1, ds(off, 512 - off)],
                            vA[:, sc, ds(h * (DH + 1), DH + 1)],
                            psb[:, ds(512 * hi + off, 512 - off)],
                            start=(sc == 0),
                            stop=(sc == n_sc - 1),
                        )

                def normalize(hp):
                    # normalize: y^T = Y_unnorm^T * (1/denom); the fast
                    # single-pass reciprocal (~18 bits) is far inside
                    # the bf16 error budget. Emission order keeps DVE
                    # from idling on the Pool broadcast round-trip.
                    ppv_A, ppv_B = ppvs[hp]
                    recs = []
                    for hi in (0, 1):
                        ppv = ppv_A if hi == 0 else ppv_B
                        den = psm.tile([1, 512], F32, tag="den")
                        nc.vector.tensor_copy(den[:], ppv[DH : DH + 1, :])
                        rec = psm.tile([1, 512], F32, tag="rec")
                        nc.vector.reciprocal_approx_fast(rec[:], den[:])
                        recs.append(rec)
                    recBs = []
                    for rec in recs:
                        recB = psm.tile([DH, 512], F32, tag="recB")
                        nc.gpsimd.partition_broadcast(recB[:], rec[:])
                        recBs.append(recB)
                    for hi, h in ((0, 2 * hp), (1, 2 * hp + 1)):
                        ppv = ppv_A if hi == 0 else ppv_B
                        nc.vector.tensor_tensor(
                            yT[ds(64 * (h % 2), DH), h // 2, ts(tj, 512)],
                            ppv[:DH, :],
                            recBs[hi][:],
                            ALU.mult,
                        )

                # the PV software pipeline is carried ACROSS the two
                # head-pair passes so the PE never drains at the seam.
                # The first `skip` steps were emitted by emit_att_pre
                # (QK/exp/mask only) back during the projection block,
                # so their exps are long done when the PVs start here.
                prev = None
                for hp in range(2):
                    if hp == 0:
                        ppvs[0] = alloc_ppv()
                        for item in pre:
                            if prev is not None:
                                emit_pv(prev)
                            prev = item
                    else:
                        ppvs[1] = alloc_ppv()
                    for sc in range(sc0[hp], n_sc):
                        kd = sc - 4 * tj  # >=0 on the causal diagonal
                        off = 128 * kd if kd > 0 else 0
                        # tj=0 is Act-overhead-bound: keep QK full width
                        # there so one full-width exp suffices (the extra
                        # columns are garbage-but-unread; PV trims at off)
                        qlo = off if tj > 0 else 0
                        pss = ps_s.tile([128, 1024], F32, tag="s", name="pss")
                        # QK^T for both heads of the pair, row-packed.
                        # Columns [0, qlo) are fully masked -> skipped.
                        for hi, (half, ppos) in enumerate(
                            [(0, (0, 0)), (512, (64, 0))]
                        ):
                            prow = slice(64 * hi, 64 * hi + 64)
                            nc.tensor.matmul(
                                pss[:, ds(half + qlo, 512 - qlo)],
                                kT[prow, hp, ts(sc, 128)],
                                qT[prow, hp, ds(512 * tj + qlo, 512 - qlo)],
                                start=True,
                                stop=True,
                                tile_position=ppos,
                            )
                        psb = pexp.tile([128, 1024], BF16, tag="p", name="psb")
                        if qlo == 0:
                            nc.scalar.activation(psb[:], pss[:], AF.Exp)
                        else:
                            # only the written windows (cols [0,qlo) of
                            # each half were skipped by the QK matmul)
                            for half in (0, 512):
                                nc.scalar.activation(
                                    psb[:, ds(half + qlo, 512 - qlo)],
                                    pss[:, ds(half + qlo, 512 - qlo)],
                                    AF.Exp,
                                )
                        if kd >= 0:
                            # causal mask on the diagonal 128-block of
                            # each head: zero p where s > t (post-exp,
                            # cheap bf16 multiply on DVE; Pool only runs
                            # partition_broadcast to avoid ucode-library
                            # swaps that head-of-line block its queue)
                            for half in (0, 512):
                                blk = psb[:, ds(half + off, 128)]
                                nc.vector.tensor_tensor(
                                    blk, blk, tri01[:], ALU.mult
                                )
                        # PV runs one step behind QK so the PE is not
                        # blocked on the exp of the current step
                        if prev is not None:
                            emit_pv(prev)
                            if prev[0] != hp:
                                normalize(prev[0])
                        prev = (hp, sc, off, psb)
                emit_pv(prev)
                normalize(1)

            def emit_o(tj):
                # o-projection for one 512-row t-block; all kk=0 matmuls
                # of a group issue before the kk=1 ones so the PE is not
                # blocked on the second pair's normalize
                for ttg in range(2):
                    tts = (2 * ttg, 2 * ttg + 1)
                    ots = {
                        tt: pout.tile([128, C], BF16, tag="o", name="ot")
                        for tt in tts
                    }
                    chains = []
                    for tt in tts:
                        t0 = 512 * tj + 128 * tt
                        for nb in range(2):
                            po = ps_s.tile([128, 512], F32, tag="s",
                                           name="po")
                            nc.tensor.matmul(
                                po[:],
                                yT[:, 0, ds(t0, 128)],
                                wo_sb[:, 0, ts(nb, 512)],
                                start=True,
                                stop=False,
                            )
                            chains.append((tt, t0, nb, po))
                    for tt, t0, nb, po in chains:
                        nc.tensor.matmul(
                            po[:],
                            yT[:, 1, ds(t0, 128)],
                            wo_sb[:, 1, ts(nb, 512)],
                            start=False,
                            stop=True,
                        )
                        if nb == 0:
                            nc.vector.tensor_copy(ots[tt][:, ts(nb, 512)],
                                                  po[:])
                        else:
                            nc.scalar.copy(ots[tt][:, ts(nb, 512)], po[:])
                    for tt in tts:
                        t0 = 512 * tj + 128 * tt
                        nc.sync.dma_start(out.ap()[ds(t0, 128), :],
                                          ots[tt][:])

            # ---- schedule: interleave per t-block ----------------------
            # warm the PE to full DVFS speed during the initial x-DMA
            # wait with dummy transposes (nothing reads the scratch)
            warm = ps_w.tile([128, 512], BF16, tag="w", name="warm")
            for _ in range(215):
                nc.tensor.transpose(warm[:, 0:128], ident[:], ident[:])
            emit_qk(0)
            pre = emit_att_pre(0, 2)
            for sc in range(4):
                emit_v(sc)
            emit_att(0, pre)
            for tj in range(1, TJ):
                emit_qk(tj)
                pre = emit_att_pre(tj, 2)
                emit_o(tj - 1)
                for sc in range(4 * tj, 4 * tj + 4):
                    emit_v(sc)
                emit_att(tj, pre)
            emit_o(TJ - 1)

    nc.compile()
    return nc


_CACHE = {}


def _get_program():
    if "nc" not in _CACHE:
        _CACHE["nc"] = build_program()
    return _CACHE["nc"]


def make_in_maps(x, wq, bq, wk, bk, wv, bv, wo):
    xb16 = np.asarray(x, np.float32).astype(NP_BF16)
    wqb = np.asarray(wq, np.float32).astype(NP_BF16)
    wkb = np.asarray(wk, np.float32).astype(NP_BF16)
    wvb = np.asarray(wv, np.float32).astype(NP_BF16)
    wob = np.asarray(wo, np.float32).astype(NP_BF16)
    in_maps = []
    for core in range(N_CORES):
        b, g = core // 4, core % 4
        sl = slice(g * DQC, (g + 1) * DQC)
        in_maps.append(
            {
                "xb": np.ascontiguousarray(xb16[b]),
                "wq": np.ascontiguousarray(wqb[:, sl]),
                "wk": np.ascontiguousarray(wkb[:, sl]),
                "wv": np.ascontiguousarray(wvb[:, sl]),
                "wo": np.ascontiguousarray(wob[sl, :]),
                "bq": np.ascontiguousarray(np.asarray(bq, np.float32)[sl]),
                "bk": np.ascontiguousarray(np.asarray(bk, np.float32)[sl]),
                "bv": np.ascontiguousarray(np.asarray(bv, np.float32)[sl]),
            }
        )
    return in_maps


def kernel(x, wq, bq, wk, bk, wv, bv, wo, bo):
    from concourse import bass_utils

    nc = _get_program()
    in_maps = make_in_maps(x, wq, bq, wk, bk, wv, bv, wo)
    res = bass_utils.run_bass_kernel_spmd(
        nc, in_maps, core_ids=list(range(N_CORES))
    )
    y = np.zeros((B, T, C), dtype=np.float32)
    for core in range(N_CORES):
        y[core // 4] += res.results[core]["out"].astype(np.float32)
    y += np.asarray(bo, np.float32)
    return y
